# revision 9
# baseline (speedup 1.0000x reference)
"""Trainium2 Bass kernel for nn_DifferentiableVectorization (8 NeuronCores).

Strategy (no collectives -- measured ~41-90us each on this runner):
  3 SPMD launches with host-side LAYOUT-ONLY glue (concat/transpose/reshape).
  L1: pool p4 channel-shard      -> pooled shard  [128(b,c), 64] per core
  L2: h1 column-shard            -> relu(flat @ iw1[:,64cols]+ib1) as [64,4]
  L3: h2, h3-slice, poly-sigmoid init, 3 grid-sample refinement steps with
      indirect-DMA 4KB patch gathers, validity MLP.

Sharding: core k in 0..7 -> batch b=k//2, point-half par=k%2 (125*4 points,
  processed as 4 groups of 125 padded to 128 partitions).
p2 is staged per batch as a patch table P[65536, 1024] f32 where row
  (y*256+x) = [f[y,x,:], f[y,xc,:], f[yc,x,:], f[yc,xc,:]] (channels-last,
  xc=min(x+1,255), yc=min(y+1,255)) -- border clamp baked in, one 4KB
  gather per sampled point. Everything fp32: grid-sample on a randn field
  amplifies coordinate error ~3e4x over 3 steps, so no low-precision
  shortcuts anywhere upstream of coordinates.
"""
import numpy as np

import concourse.bacc as bacc
import concourse.bass as bass
import concourse.mybir as mybir
import concourse.tile as tile
from concourse import bass_utils
from concourse.masks import make_identity

F32 = mybir.dt.float32
I32 = mybir.dt.int32
AF = mybir.ActivationFunctionType
OP = mybir.AluOpType

NCORES = 8
B, C, H, W = 4, 256, 256, 256
MAX_P, MAX_N = 20, 50
SCALE = 0.08
STEPS = 3
NPIX = H * W
NPG = 4          # point groups per core
GP = 125         # points per group (125*4 = 500 = half a batch's points)

# Taylor coefficients of sigmoid(x)-0.5 (odd); |x|<0.5 -> err < 1e-9
SIG_C1 = 0.25
SIG_C3 = -1.0 / 48.0
SIG_C5 = 1.0 / 480.0
SIG_C7 = -17.0 / 80640.0
SIG_C9 = 31.0 / 1451520.0

_cache = {}


# --------------------------------------------------------------------------
# L1: pooling of p4 channel shard.  in: p4s [128, 4096]  out: pool [128, 64]
# --------------------------------------------------------------------------
def build_l1():
    nc = bacc.Bacc("TRN2", target_bir_lowering=False, debug=False,
                   num_devices=NCORES)
    d_p4 = nc.dram_tensor("p4s", [128, 4096], F32, kind="ExternalInput")
    o_pool = nc.dram_tensor("o_pool", [128, 64], F32, kind="ExternalOutput")
    with tile.TileContext(nc) as tc:
        with tc.tile_pool(name="sb", bufs=1) as sb:
            t = sb.tile([128, 4096], F32)
            nc.sync.dma_start(out=t[:], in_=d_p4[:])
            t_pool = sb.tile([128, 64], F32)
            v = t[:].rearrange("p (y0 yi x0 xi) -> p y0 x0 yi xi",
                               y0=8, yi=8, x0=8, xi=8)
            nc.vector.tensor_reduce(
                out=t_pool[:].rearrange("p (a b) -> p a b", a=8, b=8),
                in_=v, op=OP.add, axis=mybir.AxisListType.XY)
            t_poolm = sb.tile([128, 64], F32)
            nc.vector.tensor_scalar_mul(out=t_poolm[:], in0=t_pool[:],
                                        scalar1=1.0 / 64.0)
            nc.sync.dma_start(out=o_pool[:], in_=t_poolm[:])
    nc.compile()
    return nc


# --------------------------------------------------------------------------
# L2: h1 column shard. in: flatT_ch [128, 512] (=[128,(c128,4b)]),
#     iw1s_ch [128, 8192] (=[128,(c128,64m)]), ib1s [64, 1]
# out: h1T slice [64, 4] (post-relu)
# --------------------------------------------------------------------------
def build_l2():
    nc = bacc.Bacc("TRN2", target_bir_lowering=False, debug=False,
                   num_devices=NCORES)
    d_flat = nc.dram_tensor("flatT_ch", [128, 512], F32, kind="ExternalInput")
    d_iw1 = nc.dram_tensor("iw1s_ch", [128, 8192], F32, kind="ExternalInput")
    d_ib1 = nc.dram_tensor("ib1s", [64, 1], F32, kind="ExternalInput")
    o_h1 = nc.dram_tensor("o_h1", [64, 4], F32, kind="ExternalOutput")
    with tile.TileContext(nc) as tc:
        with tc.tile_pool(name="sb", bufs=1) as sb, \
             tc.tile_pool(name="ps", bufs=1, space="PSUM") as pp:
            t_flat = sb.tile([128, 512], F32)
            nc.sync.dma_start(out=t_flat[:], in_=d_flat[:])
            t_w = sb.tile([128, 8192], F32)
            nc.sync.dma_start(out=t_w[:], in_=d_iw1[:])
            t_b = sb.tile([64, 1], F32)
            nc.sync.dma_start(out=t_b[:], in_=d_ib1[:])
            t_id2 = sb.tile([4, 4], F32)
            make_identity(nc, t_id2[:])
            p_h1 = pp.tile([4, 64], F32, space="PSUM")
            for c in range(128):
                nc.tensor.matmul(out=p_h1[:],
                                 lhsT=t_flat[:, 4 * c:4 * c + 4],
                                 rhs=t_w[:, 64 * c:64 * c + 64],
                                 start=(c == 0), stop=(c == 127))
            t_h1p = sb.tile([4, 64], F32)
            nc.vector.tensor_copy(out=t_h1p[:], in_=p_h1[:])
            p_h1T = pp.tile([64, 4], F32, space="PSUM", tag="tr")
            nc.tensor.transpose(out=p_h1T[:], in_=t_h1p[:],
                                identity=t_id2[0:4, 0:4])
            t_h1 = sb.tile([64, 4], F32)
            nc.scalar.activation(t_h1[:], p_h1T[:], AF.Relu, bias=t_b[:, :1])
            nc.sync.dma_start(out=o_h1[:], in_=t_h1[:])
    nc.compile()
    return nc


# --------------------------------------------------------------------------
# L3: the main kernel (per core: batch b = pid//2, half par = pid%2;
#     batch selection via host-supplied one-hot "bsel")
# --------------------------------------------------------------------------
def build_l3():
    nc = bacc.Bacc("TRN2", target_bir_lowering=False, debug=False,
                   num_devices=NCORES)
    d_h1 = nc.dram_tensor("h1T_ch", [128, 16], F32, kind="ExternalInput")
    d_iw2 = nc.dram_tensor("iw2_ch", [128, 4096], F32, kind="ExternalInput")
    d_ib2 = nc.dram_tensor("ib2", [1, 1024], F32, kind="ExternalInput")
    d_iw3 = nc.dram_tensor("iw3s_ch", [128, 8000], F32, kind="ExternalInput")
    d_ib3 = nc.dram_tensor("ib3s", [1, 1000], F32, kind="ExternalInput")
    d_rw1a = nc.dram_tensor("rw1a", [128, 256], F32, kind="ExternalInput")
    d_rw1b = nc.dram_tensor("rw1b", [128, 256], F32, kind="ExternalInput")
    d_rw1c = nc.dram_tensor("rw1c", [2, 256], F32, kind="ExternalInput")
    d_rb1a = nc.dram_tensor("rb1a", [128, 1], F32, kind="ExternalInput")
    d_rb1b = nc.dram_tensor("rb1b", [128, 1], F32, kind="ExternalInput")
    d_rw2a = nc.dram_tensor("rw2a", [128, 128], F32, kind="ExternalInput")
    d_rw2b = nc.dram_tensor("rw2b", [128, 128], F32, kind="ExternalInput")
    d_rb2 = nc.dram_tensor("rb2T", [128, 1], F32, kind="ExternalInput")
    d_rw3 = nc.dram_tensor("rw3", [128, 2], F32, kind="ExternalInput")
    d_rb3 = nc.dram_tensor("rb3T", [2, 1], F32, kind="ExternalInput")
    d_vw1 = nc.dram_tensor("vw1p", [100, 128], F32, kind="ExternalInput")
    d_vb1 = nc.dram_tensor("vb1", [1, 128], F32, kind="ExternalInput")
    d_vw2 = nc.dram_tensor("vw2", [128, 1], F32, kind="ExternalInput")
    d_vb2 = nc.dram_tensor("vb2", [1, 1], F32, kind="ExternalInput")
    d_bsel = nc.dram_tensor("bsel", [4, 1], F32, kind="ExternalInput")
    d_patch = nc.dram_tensor("patch", [NPIX, 1024], F32, kind="ExternalInput")

    o_init = nc.dram_tensor("o_init", [2, 500], F32, kind="ExternalOutput")
    o_poly = nc.dram_tensor("o_poly", [2, 500], F32, kind="ExternalOutput")
    o_val = nc.dram_tensor("o_val", [1, 10], F32, kind="ExternalOutput")

    scratch = nc.dram_tensor("scratch_poly", [2, 500], F32)

    with tile.TileContext(nc) as tc:
        with tc.tile_pool(name="sb", bufs=1) as sb, \
             tc.tile_pool(name="gpool", bufs=4) as gpool, \
             tc.tile_pool(name="ps", bufs=2, space="PSUM") as pp, \
             tc.tile_pool(name="pst", bufs=2, space="PSUM") as ppt:

            # ---------- loads ----------
            def load(name, dram, shape):
                t = sb.tile(shape, F32, tag=name)
                nc.sync.dma_start(out=t[:], in_=dram[:])
                return t

            t_h1 = load("h1", d_h1, [128, 16])
            t_iw2 = load("iw2", d_iw2, [128, 4096])
            t_ib2 = load("ib2", d_ib2, [1, 1024])
            t_iw3 = load("iw3", d_iw3, [128, 8000])
            t_ib3 = load("ib3", d_ib3, [1, 1000])
            t_rw1a = load("rw1a", d_rw1a, [128, 256])
            t_rw1b = load("rw1b", d_rw1b, [128, 256])
            t_rw1c = load("rw1c", d_rw1c, [2, 256])
            t_rb1a = load("rb1a", d_rb1a, [128, 1])
            t_rb1b = load("rb1b", d_rb1b, [128, 1])
            t_rw2a = load("rw2a", d_rw2a, [128, 128])
            t_rw2b = load("rw2b", d_rw2b, [128, 128])
            t_rb2 = load("rb2", d_rb2, [128, 1])
            t_rw3 = load("rw3", d_rw3, [128, 2])
            t_rb3 = load("rb3", d_rb3, [2, 1])
            t_vw1 = load("vw1", d_vw1, [100, 128])
            t_vb1 = load("vb1", d_vb1, [1, 128])
            t_vw2 = load("vw2", d_vw2, [128, 1])
            t_vb2 = load("vb2", d_vb2, [1, 1])
            t_bsel = load("bsel", d_bsel, [4, 1])
            t_ones = sb.tile([1, 1024], F32)
            nc.vector.memset(t_ones[:], 1.0)
            t_id = sb.tile([128, 128], F32)
            make_identity(nc, t_id[:])

            # ---------- h2 = relu(h1 @ iw2 + ib2) ----------
            iw2v = t_iw2[:].rearrange("p (c n) -> p c n", c=4)
            p_h2a = pp.tile([4, 512], F32, space="PSUM", tag="mm")
            p_h2b = pp.tile([4, 512], F32, space="PSUM", tag="mm")
            for nh, p_h2 in ((0, p_h2a), (1, p_h2b)):
                for kc in range(4):
                    nc.tensor.matmul(
                        out=p_h2[:],
                        lhsT=t_h1[:, 4 * kc:4 * kc + 4],
                        rhs=iw2v[:, kc, 512 * nh:512 * nh + 512],
                        start=(kc == 0), stop=False)
                nc.tensor.matmul(out=p_h2[:], lhsT=t_ones[:1, :4],
                                 rhs=t_ib2[:1, 512 * nh:512 * nh + 512],
                                 start=False, stop=True)
            t_h2 = sb.tile([4, 1024], F32)
            nc.scalar.activation(t_h2[:, 0:512], p_h2a[:], AF.Relu)
            nc.scalar.activation(t_h2[:, 512:1024], p_h2b[:], AF.Relu)
            # h2T [128, 32] = 8 transposes of [4,128]
            t_h2T = sb.tile([128, 32], F32)
            for c in range(8):
                p_tr = ppt.tile([128, 4], F32, space="PSUM", tag="tr")
                nc.tensor.transpose(out=p_tr[:],
                                    in_=t_h2[:, 128 * c:128 * c + 128],
                                    identity=t_id[0:4, 0:4])
                nc.vector.tensor_copy(out=t_h2T[:, 4 * c:4 * c + 4],
                                      in_=p_tr[:])

            # ---------- h3 slice + sigmoid (poly) ----------
            iw3v = t_iw3[:].rearrange("p (c n) -> p c n", c=8)
            p_h3a = pp.tile([4, 500], F32, space="PSUM", tag="mm")
            p_h3b = pp.tile([4, 500], F32, space="PSUM", tag="mm")
            for nh, p_h3 in ((0, p_h3a), (1, p_h3b)):
                for kc in range(8):
                    nc.tensor.matmul(
                        out=p_h3[:],
                        lhsT=t_h2T[:, 4 * kc:4 * kc + 4],
                        rhs=iw3v[:, kc, 500 * nh:500 * nh + 500],
                        start=(kc == 0), stop=False)
                nc.tensor.matmul(out=p_h3[:], lhsT=t_ones[:1, :4],
                                 rhs=t_ib3[:1, 500 * nh:500 * nh + 500],
                                 start=False, stop=True)
            # ---------- batch-select h3 preact, transpose to point layout,
            # sigmoid poly on [128, 8] ----------
            t_h3 = sb.tile([4, 1000], F32)
            nc.vector.tensor_copy(out=t_h3[:, 0:500], in_=p_h3a[:])
            nc.vector.tensor_copy(out=t_h3[:, 500:1000], in_=p_h3b[:])
            t_pre = sb.tile([1, 1024], F32)
            nc.vector.memset(t_pre[:], 0.0)
            for nh in range(2):
                p_sel = ppt.tile([1, 500], F32, space="PSUM", tag="tr")
                nc.tensor.matmul(out=p_sel[:], lhsT=t_bsel[:],
                                 rhs=t_h3[:, 500 * nh:500 * nh + 500],
                                 start=True, stop=True)
                nc.vector.tensor_copy(out=t_pre[:, 500 * nh:500 * nh + 500],
                                      in_=p_sel[:])
            prev = t_pre[:].rearrange("o (q t) -> o q t", t=2)
            t_prep = sb.tile([128, 8], F32)
            nc.vector.memset(t_prep[:], 0.0)
            for g in range(NPG):
                for cxy in range(2):
                    p_tr = ppt.tile([128, 1], F32, space="PSUM", tag="tr")
                    nc.tensor.transpose(
                        out=p_tr[0:125, :],
                        in_=prev[:, 125 * g:125 * g + 125, cxy],
                        identity=t_id[0:1, 0:1])
                    nc.vector.tensor_copy(
                        out=t_prep[0:125, 2 * g + cxy:2 * g + cxy + 1],
                        in_=p_tr[0:125, :])
            # sigmoid(x) = 0.5 + x*P(x^2); sigmoid(0) = 0.5 handles pad rows
            t_pall = sb.tile([128, 8], F32)
            t_x2 = sb.tile([128, 8], F32)
            nc.vector.tensor_tensor(out=t_x2[:], in0=t_prep[:], in1=t_prep[:],
                                    op=OP.mult)
            t_p = sb.tile([128, 8], F32)
            nc.vector.tensor_scalar(out=t_p[:], in0=t_x2[:], scalar1=SIG_C9,
                                    scalar2=SIG_C7, op0=OP.mult, op1=OP.add)
            for cc in (SIG_C5, SIG_C3, SIG_C1):
                nc.vector.tensor_tensor(out=t_p[:], in0=t_p[:], in1=t_x2[:],
                                        op=OP.mult)
                nc.vector.tensor_scalar(out=t_p[:], in0=t_p[:], scalar1=cc,
                                        scalar2=None, op0=OP.add)
            nc.vector.tensor_tensor(out=t_p[:], in0=t_p[:], in1=t_prep[:],
                                    op=OP.mult)
            nc.vector.tensor_scalar(out=t_pall[:], in0=t_p[:],
                                    scalar1=0.5, scalar2=None, op0=OP.add)
            # pT [2, 512] from p_all; o_init = pT points (host de-interleaves)
            t_pT = sb.tile([2, 512], F32)
            nc.vector.memset(t_pT[:], 0.5)
            for g in range(NPG):
                p_tr = ppt.tile([2, 128], F32, space="PSUM", tag="tr")
                nc.tensor.transpose(out=p_tr[:],
                                    in_=t_pall[:, 2 * g:2 * g + 2],
                                    identity=t_id[:])
                nc.vector.tensor_copy(out=t_pT[:, 128 * g:128 * g + 128],
                                      in_=p_tr[:])
            initT_view = t_pT[:].rearrange("p (g q) -> p g q", g=4)[:, :, 0:125]
            nc.sync.dma_start(out=o_init[:], in_=initT_view)

            # ---------- refinement loop ----------
            for step in range(STEPS):
                # ix = clip(((c*2-1+1)*256-1)*0.5, 0, 255), ref op order
                t_u = sb.tile([128, 8], F32, tag="cm1")
                nc.vector.tensor_scalar(out=t_u[:], in0=t_pall[:],
                                        scalar1=2.0, scalar2=1.0,
                                        op0=OP.mult, op1=OP.subtract)
                t_v = sb.tile([128, 8], F32, tag="cm2")
                nc.vector.tensor_scalar(out=t_v[:], in0=t_u[:], scalar1=1.0,
                                        scalar2=None, op0=OP.add)
                t_w = sb.tile([128, 8], F32, tag="cm3")
                nc.vector.tensor_scalar(out=t_w[:], in0=t_v[:],
                                        scalar1=256.0, scalar2=1.0,
                                        op0=OP.mult, op1=OP.subtract)
                t_ix = sb.tile([128, 8], F32, tag="cm4")
                nc.vector.tensor_scalar(out=t_ix[:], in0=t_w[:], scalar1=0.5,
                                        scalar2=None, op0=OP.mult)
                nc.vector.tensor_scalar(out=t_ix[:], in0=t_ix[:],
                                        scalar1=0.0, scalar2=255.0,
                                        op0=OP.max, op1=OP.min)
                t_ri = sb.tile([128, 8], I32, tag="cm5")
                nc.vector.tensor_copy(out=t_ri[:], in_=t_ix[:])
                t_rf = sb.tile([128, 8], F32, tag="cm6")
                nc.vector.tensor_copy(out=t_rf[:], in_=t_ri[:])
                t_gt = sb.tile([128, 8], F32, tag="cm7")
                nc.vector.tensor_tensor(out=t_gt[:], in0=t_rf[:], in1=t_ix[:],
                                        op=OP.is_gt)
                t_fl = sb.tile([128, 8], F32, tag="cm8")
                nc.vector.tensor_tensor(out=t_fl[:], in0=t_rf[:], in1=t_gt[:],
                                        op=OP.subtract)
                t_wf = sb.tile([128, 8], F32, tag="cm9")
                nc.vector.tensor_tensor(out=t_wf[:], in0=t_ix[:], in1=t_fl[:],
                                        op=OP.subtract)
                t_om = sb.tile([128, 8], F32, tag="cm10")
                nc.vector.tensor_scalar(out=t_om[:], in0=t_wf[:],
                                        scalar1=-1.0, scalar2=1.0,
                                        op0=OP.mult, op1=OP.add)
                flv = t_fl[:].rearrange("p (g t) -> p g t", t=2)
                t_idxf = sb.tile([128, 4], F32, tag="cm11")
                nc.vector.tensor_scalar(out=t_idxf[:], in0=flv[:, :, 1],
                                        scalar1=256.0, scalar2=None,
                                        op0=OP.mult)
                nc.vector.tensor_tensor(out=t_idxf[:], in0=t_idxf[:],
                                        in1=flv[:, :, 0], op=OP.add)
                t_idx = sb.tile([128, 4], I32, tag="cm12")
                nc.vector.tensor_copy(out=t_idx[:], in_=t_idxf[:])

                # corner weights cw[p, 4g+c], c in (00,01,10,11):
                # w00=omx*omy, w01=wx*omy, w10=omx*wy, w11=wx*wy
                wfv = t_wf[:].rearrange("p (g t) -> p g t", t=2)
                omv = t_om[:].rearrange("p (g t) -> p g t", t=2)
                t_cw = sb.tile([128, 16], F32, tag="cw")
                cwv = t_cw[:].rearrange("p (g c) -> p c g", c=4)
                for c, (xp, yp) in enumerate(((omv, omv), (wfv, omv),
                                              (omv, wfv), (wfv, wfv))):
                    nc.vector.tensor_tensor(out=cwv[:, c, :],
                                            in0=xp[:, :, 0], in1=yp[:, :, 1],
                                            op=OP.mult)
                inpT_a = sb.tile([128, 512], F32, tag="inpa")
                inpT_b = sb.tile([128, 512], F32, tag="inpb")
                for g in range(NPG):
                    t_g = gpool.tile([128, 1024], F32, tag="gath")
                    nc.gpsimd.indirect_dma_start(
                        out=t_g[:], out_offset=None, in_=d_patch[:],
                        in_offset=bass.IndirectOffsetOnAxis(
                            ap=t_idx[:, g:g + 1], axis=0))
                    # m_c = f_c * w_c ; sampled = (m0+m1) + (m2+m3)
                    t_m = gpool.tile([128, 1024], F32, tag="m")
                    for blk in range(4):
                        nc.vector.tensor_scalar(
                            out=t_m[:, 256 * blk:256 * blk + 256],
                            in0=t_g[:, 256 * blk:256 * blk + 256],
                            scalar1=t_cw[:, 4 * g + blk:4 * g + blk + 1],
                            scalar2=None, op0=OP.mult)
                    t_tb = gpool.tile([128, 512], F32, tag="tb")
                    nc.vector.tensor_tensor(
                        out=t_tb[:].rearrange("p (a b) -> p a b", a=2),
                        in0=t_m[:].rearrange("p (a b) -> p a b", a=2)[:, :, 0:256],
                        in1=t_m[:].rearrange("p (a b) -> p a b", a=2)[:, :, 256:512],
                        op=OP.add)
                    t_samp = gpool.tile([128, 256], F32, tag="samp")
                    nc.vector.tensor_tensor(out=t_samp[:], in0=t_tb[:, 0:256],
                                            in1=t_tb[:, 256:512], op=OP.add)
                    for hc, dest in ((0, inpT_a), (1, inpT_b)):
                        p_tr = ppt.tile([128, 128], F32, space="PSUM",
                                        tag="tr")
                        nc.tensor.transpose(
                            out=p_tr[:],
                            in_=t_samp[:, 128 * hc:128 * hc + 128],
                            identity=t_id[:])
                        nc.vector.tensor_copy(
                            out=dest[:, 128 * g:128 * g + 128], in_=p_tr[:])

                # mm1: r1T (2 M-chunks of [128, 512])
                r1Ts = []
                for mh, (r1tag, rb1) in enumerate((("r1Ta", t_rb1a),
                                                   ("r1Tb", t_rb1b))):
                    p_r1 = pp.tile([128, 512], F32, space="PSUM", tag="mm")
                    nc.tensor.matmul(out=p_r1[:],
                                     lhsT=t_rw1a[:, 128 * mh:128 * mh + 128],
                                     rhs=inpT_a[:], start=True, stop=False)
                    nc.tensor.matmul(out=p_r1[:],
                                     lhsT=t_rw1b[:, 128 * mh:128 * mh + 128],
                                     rhs=inpT_b[:], start=False, stop=False)
                    nc.tensor.matmul(out=p_r1[:],
                                     lhsT=t_rw1c[:, 128 * mh:128 * mh + 128],
                                     rhs=t_pT[:], start=False, stop=True)
                    r1T = sb.tile([128, 512], F32, tag=r1tag)
                    nc.scalar.activation(r1T[:], p_r1[:], AF.Relu,
                                         bias=rb1[:, :1])
                    r1Ts.append(r1T)
                # mm2: r2T [128, 512]
                p_r2 = pp.tile([128, 512], F32, space="PSUM", tag="mm")
                nc.tensor.matmul(out=p_r2[:], lhsT=t_rw2a[:], rhs=r1Ts[0][:],
                                 start=True, stop=False)
                nc.tensor.matmul(out=p_r2[:], lhsT=t_rw2b[:], rhs=r1Ts[1][:],
                                 start=False, stop=True)
                r2T = sb.tile([128, 512], F32, tag="r2T")
                nc.scalar.activation(r2T[:], p_r2[:], AF.Relu,
                                     bias=t_rb2[:, :1])
                # mm3: disp [2, 512]
                p_r3 = pp.tile([2, 512], F32, space="PSUM", tag="mm")
                nc.tensor.matmul(out=p_r3[:], lhsT=t_rw3[:], rhs=r2T[:],
                                 start=True, stop=True)
                t_th = sb.tile([2, 512], F32, tag="th")
                nc.scalar.activation(t_th[:], p_r3[:], AF.Tanh,
                                     bias=t_rb3[:, :1])
                t_disp = sb.tile([2, 512], F32, tag="disp")
                nc.vector.tensor_scalar(out=t_disp[:], in0=t_th[:],
                                        scalar1=SCALE, scalar2=None,
                                        op0=OP.mult)
                if step < STEPS - 1:
                    # transpose disp to point layout, update p_all on DVE
                    # (fast path for next step's coord math)
                    t_dp = sb.tile([128, 8], F32, tag="dp")
                    for g in range(NPG):
                        p_tr = ppt.tile([128, 2], F32, space="PSUM", tag="tr")
                        nc.tensor.transpose(
                            out=p_tr[:], in_=t_disp[:, 128 * g:128 * g + 128],
                            identity=t_id[0:2, 0:2])
                        nc.vector.tensor_copy(out=t_dp[:, 2 * g:2 * g + 2],
                                              in_=p_tr[:])
                    nc.vector.tensor_tensor(out=t_pall[:], in0=t_pall[:],
                                            in1=t_dp[:], op=OP.add)
                    nc.vector.tensor_scalar(out=t_pall[:], in0=t_pall[:],
                                            scalar1=0.0, scalar2=1.0,
                                            op0=OP.max, op1=OP.min)
                # pT = clip(pT + disp, 0, 1) on gpsimd (off critical path)
                nc.gpsimd.tensor_tensor(out=t_pT[:], in0=t_pT[:],
                                        in1=t_disp[:], op=OP.add)
                nc.gpsimd.tensor_scalar(out=t_pT[:], in0=t_pT[:],
                                        scalar1=0.0, scalar2=1.0,
                                        op0=OP.max, op1=OP.min)

            # ---------- outputs ----------
            pT_view = t_pT[:].rearrange("p (g q) -> p g q", g=4)[:, :, 0:125]
            nc.sync.dma_start(out=o_poly[:], in_=pT_view)
            nc.sync.dma_start(out=scratch[:], in_=pT_view)
            # validity: polyfT [100, 10] via c-major bounce + transpose
            t_pf = sb.tile([10, 100], F32)
            nc.sync.dma_start(
                out=t_pf[:].rearrange("p (c n) -> p c n", c=2),
                in_=scratch[:].rearrange("c (p n) -> p c n", p=10))
            p_pfT = ppt.tile([100, 10], F32, space="PSUM", tag="tr")
            nc.tensor.transpose(out=p_pfT[:], in_=t_pf[:],
                                identity=t_id[0:10, 0:10])
            t_pfT = sb.tile([100, 10], F32)
            nc.vector.tensor_copy(out=t_pfT[:], in_=p_pfT[:])
            p_v1 = pp.tile([128, 10], F32, space="PSUM", tag="mm")
            nc.tensor.matmul(out=p_v1[:], lhsT=t_vw1[:], rhs=t_pfT[:],
                             start=True, stop=False)
            nc.tensor.matmul(out=p_v1[:], lhsT=t_vb1[:1, :],
                             rhs=t_ones[:1, 0:10], start=False, stop=True)
            t_v1 = sb.tile([128, 10], F32)
            nc.scalar.activation(t_v1[:], p_v1[:], AF.Relu)
            p_v2 = pp.tile([1, 10], F32, space="PSUM", tag="mm")
            nc.tensor.matmul(out=p_v2[:], lhsT=t_vw2[:], rhs=t_v1[:],
                             start=True, stop=False)
            nc.tensor.matmul(out=p_v2[:], lhsT=t_vb2[:1, :],
                             rhs=t_ones[:1, 0:10], start=False, stop=True)
            t_val = sb.tile([1, 10], F32)
            nc.scalar.activation(t_val[:], p_v2[:], AF.Sigmoid)
            nc.sync.dma_start(out=o_val[:], in_=t_val[:])
    nc.compile()
    return nc


# --------------------------------------------------------------------------
# host-side layout helpers (pure data movement, no arithmetic)
# --------------------------------------------------------------------------
def _build_patch_tables(p2):
    """p2 [4, C, H, W] -> per-batch [65536, 1024] f32:
    row (y*W+x) = [f(y,x), f(y,xc), f(yc,x), f(yc,xc)] channels-last."""
    out = []
    for b in range(B):
        hwc = np.ascontiguousarray(p2[b].transpose(1, 2, 0))  # [H, W, C]
        xc = np.concatenate([hwc[:, 1:, :], hwc[:, -1:, :]], axis=1)
        yc = np.concatenate([hwc[1:, :, :], hwc[-1:, :, :]], axis=0)
        ycxc = np.concatenate([xc[1:, :, :], xc[-1:, :, :]], axis=0)
        tab = np.concatenate([hwc, xc, yc, ycxc], axis=2)  # [H, W, 1024]
        out.append(np.ascontiguousarray(tab.reshape(NPIX, 1024)))
    return out


def _chunk_rows(a, p=128):
    """[K, N] -> [p, (K//p)*N], partition-major chunks for matmul operands."""
    K, N = a.shape
    c = K // p
    return np.ascontiguousarray(
        a.reshape(c, p, N).transpose(1, 0, 2).reshape(p, c * N))


def kernel(**inputs):
    f32 = lambda k: np.asarray(inputs[k], np.float32)
    p2, p4 = f32("p2"), f32("p4")
    iw1, ib1, iw2, ib2, iw3, ib3 = (f32(k) for k in
                                    ("iw1", "ib1", "iw2", "ib2", "iw3", "ib3"))
    rw1, rb1, rw2, rb2, rw3, rb3 = (f32(k) for k in
                                    ("rw1", "rb1", "rw2", "rb2", "rw3", "rb3"))
    vw1, vb1, vw2, vb2 = (f32(k) for k in ("vw1", "vb1", "vw2", "vb2"))

    if "l1" not in _cache:
        _cache["l1"] = build_l1()
        _cache["l2"] = build_l2()
        _cache["l3"] = build_l3()
    cores = list(range(NCORES))
    exec_times = []

    # ---- L1: pooling ----
    in1 = [{"p4s": np.ascontiguousarray(
        p4[:, 32 * k:32 * k + 32].reshape(128, 4096))} for k in range(NCORES)]
    r1 = bass_utils.run_bass_kernel_spmd(_cache["l1"], in1, core_ids=cores,
                                         **_trace_kw())
    exec_times.append(r1.exec_time_ns)
    pooled = np.concatenate([r1.results[k]["o_pool"].reshape(4, 32, 64)
                             for k in range(NCORES)], axis=1)  # [4,256,64]
    flatT = np.ascontiguousarray(pooled.reshape(4, 16384).T)   # [16384,4]
    flatT_ch = _chunk_rows(flatT)                              # [128,512]

    # ---- L2: h1 slices ----
    in2 = [{
        "flatT_ch": flatT_ch,
        "iw1s_ch": _chunk_rows(np.ascontiguousarray(
            iw1[:, 64 * k:64 * k + 64])),
        "ib1s": np.ascontiguousarray(ib1[64 * k:64 * k + 64].reshape(64, 1)),
    } for k in range(NCORES)]
    r2 = bass_utils.run_bass_kernel_spmd(_cache["l2"], in2, core_ids=cores,
                                         **_trace_kw())
    exec_times.append(r2.exec_time_ns)
    h1T = np.concatenate([r2.results[k]["o_h1"] for k in range(NCORES)],
                         axis=0)                               # [512, 4]
    h1T_ch = _chunk_rows(h1T)                                  # [128, 16]

    # ---- L3: main ----
    patches = _build_patch_tables(p2)
    vw1p = np.ascontiguousarray(
        vw1.reshape(50, 2, 128).transpose(1, 0, 2).reshape(100, 128))
    common = {
        "h1T_ch": h1T_ch,
        "iw2_ch": _chunk_rows(iw2),
        "ib2": ib2.reshape(1, 1024),
        "rw1a": np.ascontiguousarray(rw1[0:128]),
        "rw1b": np.ascontiguousarray(rw1[128:256]),
        "rw1c": np.ascontiguousarray(rw1[256:258]),
        "rb1a": np.ascontiguousarray(rb1[0:128].reshape(128, 1)),
        "rb1b": np.ascontiguousarray(rb1[128:256].reshape(128, 1)),
        "rw2a": np.ascontiguousarray(rw2[0:128]),
        "rw2b": np.ascontiguousarray(rw2[128:256]),
        "rb2T": rb2.reshape(128, 1),
        "rw3": rw3, "rb3T": rb3.reshape(2, 1),
        "vw1p": vw1p, "vb1": vb1.reshape(1, 128),
        "vw2": vw2, "vb2": vb2.reshape(1, 1),
    }
    in3 = []
    for k in range(NCORES):
        b, par = k // 2, k % 2
        m = dict(common)
        m["iw3s_ch"] = _chunk_rows(np.ascontiguousarray(
            iw3[:, 1000 * par:1000 * par + 1000]))
        m["ib3s"] = np.ascontiguousarray(
            ib3[1000 * par:1000 * par + 1000].reshape(1, 1000))
        m["bsel"] = np.eye(4, dtype=np.float32)[:, b:b + 1]
        m["patch"] = patches[b]
        in3.append(m)
    r3 = bass_utils.run_bass_kernel_spmd(_cache["l3"], in3, core_ids=cores,
                                         **_trace_kw())
    exec_times.append(r3.exec_time_ns)

    polygons = np.zeros((B, MAX_P, MAX_N, 2), np.float32)
    validity = np.zeros((B, MAX_P), np.float32)
    init_p = np.zeros((B, MAX_P, MAX_N, 2), np.float32)
    for k in range(NCORES):
        b, par = k // 2, k % 2
        o = r3.results[k]
        init_p[b, 10 * par:10 * par + 10] = \
            np.ascontiguousarray(o["o_init"].T).reshape(10, 50, 2)
        polygons[b, 10 * par:10 * par + 10] = \
            np.ascontiguousarray(o["o_poly"].T).reshape(10, 50, 2)
        validity[b, 10 * par:10 * par + 10] = o["o_val"][0]

    kernel.last_exec_times = exec_times
    kernel.last_results = (r1, r2, r3)
    return polygons, validity, init_p


kernel.last_exec_times = []
kernel.last_results = None
_TRACE = {"on": False}


def _trace_kw():
    return {"trace": True} if _TRACE["on"] else {}


def enable_trace():
    """Used by test.py; requires the NTFF hook (see hwprof)."""
    _TRACE["on"] = True


# revision 10
# speedup vs baseline: 1.0300x; 1.0300x over previous
"""Trainium2 Bass kernel for nn_DifferentiableVectorization (8 NeuronCores).

Strategy (no collectives -- measured ~41-90us each on this runner):
  3 SPMD launches with host-side LAYOUT-ONLY glue (concat/transpose/reshape).
  L1: pool p4 channel-shard      -> pooled shard  [128(b,c), 64] per core
  L2: h1 column-shard            -> relu(flat @ iw1[:,64cols]+ib1) as [64,4]
  L3: h2, h3-slice, poly-sigmoid init, 3 grid-sample refinement steps with
      indirect-DMA 4KB patch gathers, validity MLP.

Sharding: core k in 0..7 -> batch b=k//2, point-half par=k%2 (125*4 points,
  processed as 4 groups of 125 padded to 128 partitions).
p2 is staged per batch as a patch table P[65536, 1024] f32 where row
  (y*256+x) = [f[y,x,:], f[y,xc,:], f[yc,x,:], f[yc,xc,:]] (channels-last,
  xc=min(x+1,255), yc=min(y+1,255)) -- border clamp baked in, one 4KB
  gather per sampled point. Everything fp32: grid-sample on a randn field
  amplifies coordinate error ~3e4x over 3 steps, so no low-precision
  shortcuts anywhere upstream of coordinates.
"""
import numpy as np

import concourse.bacc as bacc
import concourse.bass as bass
import concourse.mybir as mybir
import concourse.tile as tile
from concourse import bass_utils
from concourse.masks import make_identity

F32 = mybir.dt.float32
I32 = mybir.dt.int32
AF = mybir.ActivationFunctionType
OP = mybir.AluOpType

NCORES = 8
B, C, H, W = 4, 256, 256, 256
MAX_P, MAX_N = 20, 50
SCALE = 0.08
STEPS = 3
NPIX = H * W
NPG = 4          # point groups per core
GP = 125         # points per group (125*4 = 500 = half a batch's points)

# Taylor coefficients of sigmoid(x)-0.5 (odd); |x|<0.5 -> err < 1e-9
SIG_C1 = 0.25
SIG_C3 = -1.0 / 48.0
SIG_C5 = 1.0 / 480.0
SIG_C7 = -17.0 / 80640.0
SIG_C9 = 31.0 / 1451520.0

_cache = {}


# --------------------------------------------------------------------------
# L1: pooling of p4 channel shard.  in: p4s [128, 4096]  out: pool [128, 64]
# --------------------------------------------------------------------------
def build_l1():
    nc = bacc.Bacc("TRN2", target_bir_lowering=False, debug=False,
                   num_devices=NCORES)
    d_p4 = nc.dram_tensor("p4s", [128, 4096], F32, kind="ExternalInput")
    o_pool = nc.dram_tensor("o_pool", [128, 64], F32, kind="ExternalOutput")
    with tile.TileContext(nc) as tc:
        with tc.tile_pool(name="sb", bufs=1) as sb:
            t = sb.tile([128, 4096], F32)
            nc.sync.dma_start(out=t[:], in_=d_p4[:])
            t_pool = sb.tile([128, 64], F32)
            v = t[:].rearrange("p (y0 yi x0 xi) -> p y0 x0 yi xi",
                               y0=8, yi=8, x0=8, xi=8)
            nc.vector.tensor_reduce(
                out=t_pool[:].rearrange("p (a b) -> p a b", a=8, b=8),
                in_=v, op=OP.add, axis=mybir.AxisListType.XY)
            t_poolm = sb.tile([128, 64], F32)
            nc.vector.tensor_scalar_mul(out=t_poolm[:], in0=t_pool[:],
                                        scalar1=1.0 / 64.0)
            nc.sync.dma_start(out=o_pool[:], in_=t_poolm[:])
    nc.compile()
    return nc


# --------------------------------------------------------------------------
# L2: h1 column shard. in: flatT_ch [128, 512] (=[128,(c128,4b)]),
#     iw1s_ch [128, 8192] (=[128,(c128,64m)]), ib1s [64, 1]
# out: h1T slice [64, 4] (post-relu)
# --------------------------------------------------------------------------
def build_l2():
    nc = bacc.Bacc("TRN2", target_bir_lowering=False, debug=False,
                   num_devices=NCORES)
    d_flat = nc.dram_tensor("flatT_ch", [128, 512], F32, kind="ExternalInput")
    d_iw1 = nc.dram_tensor("iw1s_ch", [128, 8192], F32, kind="ExternalInput")
    d_ib1 = nc.dram_tensor("ib1s", [64, 1], F32, kind="ExternalInput")
    o_h1 = nc.dram_tensor("o_h1", [64, 4], F32, kind="ExternalOutput")
    with tile.TileContext(nc) as tc:
        with tc.tile_pool(name="sb", bufs=1) as sb, \
             tc.tile_pool(name="ps", bufs=1, space="PSUM") as pp:
            t_flat = sb.tile([128, 512], F32)
            nc.sync.dma_start(out=t_flat[:], in_=d_flat[:])
            t_w = sb.tile([128, 8192], F32)
            nc.sync.dma_start(out=t_w[:], in_=d_iw1[:])
            t_b = sb.tile([64, 1], F32)
            nc.sync.dma_start(out=t_b[:], in_=d_ib1[:])
            t_id2 = sb.tile([4, 4], F32)
            make_identity(nc, t_id2[:])
            p_h1 = pp.tile([4, 64], F32, space="PSUM")
            for c in range(128):
                nc.tensor.matmul(out=p_h1[:],
                                 lhsT=t_flat[:, 4 * c:4 * c + 4],
                                 rhs=t_w[:, 64 * c:64 * c + 64],
                                 start=(c == 0), stop=(c == 127))
            t_h1p = sb.tile([4, 64], F32)
            nc.vector.tensor_copy(out=t_h1p[:], in_=p_h1[:])
            p_h1T = pp.tile([64, 4], F32, space="PSUM", tag="tr")
            nc.tensor.transpose(out=p_h1T[:], in_=t_h1p[:],
                                identity=t_id2[0:4, 0:4])
            t_h1 = sb.tile([64, 4], F32)
            nc.scalar.activation(t_h1[:], p_h1T[:], AF.Relu, bias=t_b[:, :1])
            nc.sync.dma_start(out=o_h1[:], in_=t_h1[:])
    nc.compile()
    return nc


# --------------------------------------------------------------------------
# L3: the main kernel (per core: batch b = pid//2, half par = pid%2;
#     batch selection via host-supplied one-hot "bsel")
# --------------------------------------------------------------------------
def build_l3():
    nc = bacc.Bacc("TRN2", target_bir_lowering=False, debug=False,
                   num_devices=NCORES)
    d_h1 = nc.dram_tensor("h1T_ch", [128, 16], F32, kind="ExternalInput")
    d_iw2 = nc.dram_tensor("iw2_ch", [128, 4096], F32, kind="ExternalInput")
    d_ib2 = nc.dram_tensor("ib2", [1, 1024], F32, kind="ExternalInput")
    d_iw3 = nc.dram_tensor("iw3s_ch", [128, 8000], F32, kind="ExternalInput")
    d_ib3 = nc.dram_tensor("ib3s", [1, 1000], F32, kind="ExternalInput")
    d_rw1a = nc.dram_tensor("rw1a", [128, 256], F32, kind="ExternalInput")
    d_rw1b = nc.dram_tensor("rw1b", [128, 256], F32, kind="ExternalInput")
    d_rw1c = nc.dram_tensor("rw1c", [2, 256], F32, kind="ExternalInput")
    d_rb1a = nc.dram_tensor("rb1a", [128, 1], F32, kind="ExternalInput")
    d_rb1b = nc.dram_tensor("rb1b", [128, 1], F32, kind="ExternalInput")
    d_rw2a = nc.dram_tensor("rw2a", [128, 128], F32, kind="ExternalInput")
    d_rw2b = nc.dram_tensor("rw2b", [128, 128], F32, kind="ExternalInput")
    d_rb2 = nc.dram_tensor("rb2T", [128, 1], F32, kind="ExternalInput")
    d_rw3 = nc.dram_tensor("rw3", [128, 2], F32, kind="ExternalInput")
    d_rb3 = nc.dram_tensor("rb3T", [2, 1], F32, kind="ExternalInput")
    d_vw1 = nc.dram_tensor("vw1p", [100, 128], F32, kind="ExternalInput")
    d_vb1 = nc.dram_tensor("vb1", [1, 128], F32, kind="ExternalInput")
    d_vw2 = nc.dram_tensor("vw2", [128, 1], F32, kind="ExternalInput")
    d_vb2 = nc.dram_tensor("vb2", [1, 1], F32, kind="ExternalInput")
    d_bsel = nc.dram_tensor("bsel", [4, 1], F32, kind="ExternalInput")
    d_patch = nc.dram_tensor("patch", [NPIX, 1024], F32, kind="ExternalInput")

    o_init = nc.dram_tensor("o_init", [2, 500], F32, kind="ExternalOutput")
    o_poly = nc.dram_tensor("o_poly", [2, 500], F32, kind="ExternalOutput")
    o_val = nc.dram_tensor("o_val", [1, 10], F32, kind="ExternalOutput")

    scratch = nc.dram_tensor("scratch_poly", [2, 500], F32)

    with tile.TileContext(nc) as tc:
        with tc.tile_pool(name="sb", bufs=1) as sb, \
             tc.tile_pool(name="gpool", bufs=4) as gpool, \
             tc.tile_pool(name="ps", bufs=2, space="PSUM") as pp, \
             tc.tile_pool(name="pst", bufs=2, space="PSUM") as ppt:

            # ---------- loads ----------
            def load(name, dram, shape):
                t = sb.tile(shape, F32, tag=name)
                nc.sync.dma_start(out=t[:], in_=dram[:])
                return t

            t_h1 = load("h1", d_h1, [128, 16])
            t_iw2 = load("iw2", d_iw2, [128, 4096])
            t_ib2 = load("ib2", d_ib2, [1, 1024])
            t_iw3 = load("iw3", d_iw3, [128, 8000])
            t_ib3 = load("ib3", d_ib3, [1, 1000])
            t_rw1a = load("rw1a", d_rw1a, [128, 256])
            t_rw1b = load("rw1b", d_rw1b, [128, 256])
            t_rw1c = load("rw1c", d_rw1c, [2, 256])
            t_rb1a = load("rb1a", d_rb1a, [128, 1])
            t_rb1b = load("rb1b", d_rb1b, [128, 1])
            t_rw2a = load("rw2a", d_rw2a, [128, 128])
            t_rw2b = load("rw2b", d_rw2b, [128, 128])
            t_rb2 = load("rb2", d_rb2, [128, 1])
            t_rw3 = load("rw3", d_rw3, [128, 2])
            t_rb3 = load("rb3", d_rb3, [2, 1])
            t_vw1 = load("vw1", d_vw1, [100, 128])
            t_vb1 = load("vb1", d_vb1, [1, 128])
            t_vw2 = load("vw2", d_vw2, [128, 1])
            t_vb2 = load("vb2", d_vb2, [1, 1])
            t_bsel = load("bsel", d_bsel, [4, 1])
            t_ones = sb.tile([1, 1024], F32)
            nc.vector.memset(t_ones[:], 1.0)
            t_id = sb.tile([128, 128], F32)
            make_identity(nc, t_id[:])

            # ---------- h2 = relu(h1 @ iw2 + ib2) ----------
            iw2v = t_iw2[:].rearrange("p (c n) -> p c n", c=4)
            p_h2a = pp.tile([4, 512], F32, space="PSUM", tag="mm")
            p_h2b = pp.tile([4, 512], F32, space="PSUM", tag="mm")
            for nh, p_h2 in ((0, p_h2a), (1, p_h2b)):
                for kc in range(4):
                    nc.tensor.matmul(
                        out=p_h2[:],
                        lhsT=t_h1[:, 4 * kc:4 * kc + 4],
                        rhs=iw2v[:, kc, 512 * nh:512 * nh + 512],
                        start=(kc == 0), stop=False)
                nc.tensor.matmul(out=p_h2[:], lhsT=t_ones[:1, :4],
                                 rhs=t_ib2[:1, 512 * nh:512 * nh + 512],
                                 start=False, stop=True)
            t_h2 = sb.tile([4, 1024], F32)
            nc.scalar.activation(t_h2[:, 0:512], p_h2a[:], AF.Relu)
            nc.scalar.activation(t_h2[:, 512:1024], p_h2b[:], AF.Relu)
            # h2T [128, 32] = 8 transposes of [4,128]
            t_h2T = sb.tile([128, 32], F32)
            for c in range(8):
                p_tr = ppt.tile([128, 4], F32, space="PSUM", tag="tr")
                nc.tensor.transpose(out=p_tr[:],
                                    in_=t_h2[:, 128 * c:128 * c + 128],
                                    identity=t_id[0:4, 0:4])
                nc.vector.tensor_copy(out=t_h2T[:, 4 * c:4 * c + 4],
                                      in_=p_tr[:])

            # ---------- h3 slice + sigmoid (poly) ----------
            iw3v = t_iw3[:].rearrange("p (c n) -> p c n", c=8)
            p_h3a = pp.tile([4, 500], F32, space="PSUM", tag="mm")
            p_h3b = pp.tile([4, 500], F32, space="PSUM", tag="mm")
            for nh, p_h3 in ((0, p_h3a), (1, p_h3b)):
                for kc in range(8):
                    nc.tensor.matmul(
                        out=p_h3[:],
                        lhsT=t_h2T[:, 4 * kc:4 * kc + 4],
                        rhs=iw3v[:, kc, 500 * nh:500 * nh + 500],
                        start=(kc == 0), stop=False)
                nc.tensor.matmul(out=p_h3[:], lhsT=t_ones[:1, :4],
                                 rhs=t_ib3[:1, 500 * nh:500 * nh + 500],
                                 start=False, stop=True)
            # ---------- batch-select h3 preact, transpose to point layout,
            # sigmoid poly on [128, 8] ----------
            t_h3 = sb.tile([4, 1000], F32)
            nc.vector.tensor_copy(out=t_h3[:, 0:500], in_=p_h3a[:])
            nc.vector.tensor_copy(out=t_h3[:, 500:1000], in_=p_h3b[:])
            t_pre = sb.tile([1, 1024], F32)
            nc.vector.memset(t_pre[:], 0.0)
            for nh in range(2):
                p_sel = ppt.tile([1, 500], F32, space="PSUM", tag="tr")
                nc.tensor.matmul(out=p_sel[:], lhsT=t_bsel[:],
                                 rhs=t_h3[:, 500 * nh:500 * nh + 500],
                                 start=True, stop=True)
                nc.vector.tensor_copy(out=t_pre[:, 500 * nh:500 * nh + 500],
                                      in_=p_sel[:])
            prev = t_pre[:].rearrange("o (q t) -> o q t", t=2)
            t_prep = sb.tile([128, 8], F32)
            nc.vector.memset(t_prep[:], 0.0)
            for g in range(NPG):
                for cxy in range(2):
                    p_tr = ppt.tile([128, 1], F32, space="PSUM", tag="tr")
                    nc.tensor.transpose(
                        out=p_tr[0:125, :],
                        in_=prev[:, 125 * g:125 * g + 125, cxy],
                        identity=t_id[0:1, 0:1])
                    nc.vector.tensor_copy(
                        out=t_prep[0:125, 2 * g + cxy:2 * g + cxy + 1],
                        in_=p_tr[0:125, :])
            # sigmoid(x) = 0.5 + x*P(x^2); sigmoid(0) = 0.5 handles pad rows
            t_pall = sb.tile([128, 8], F32)
            t_x2 = sb.tile([128, 8], F32)
            nc.vector.tensor_tensor(out=t_x2[:], in0=t_prep[:], in1=t_prep[:],
                                    op=OP.mult)
            t_p = sb.tile([128, 8], F32)
            nc.vector.tensor_scalar(out=t_p[:], in0=t_x2[:], scalar1=SIG_C9,
                                    scalar2=SIG_C7, op0=OP.mult, op1=OP.add)
            for cc in (SIG_C5, SIG_C3, SIG_C1):
                nc.vector.tensor_tensor(out=t_p[:], in0=t_p[:], in1=t_x2[:],
                                        op=OP.mult)
                nc.vector.tensor_scalar(out=t_p[:], in0=t_p[:], scalar1=cc,
                                        scalar2=None, op0=OP.add)
            nc.vector.tensor_tensor(out=t_p[:], in0=t_p[:], in1=t_prep[:],
                                    op=OP.mult)
            nc.vector.tensor_scalar(out=t_pall[:], in0=t_p[:],
                                    scalar1=0.5, scalar2=None, op0=OP.add)
            # pT [2, 512] from p_all; o_init = pT points (host de-interleaves)
            t_pT = sb.tile([2, 512], F32)
            nc.vector.memset(t_pT[:], 0.5)
            for g in range(NPG):
                p_tr = ppt.tile([2, 128], F32, space="PSUM", tag="tr")
                nc.tensor.transpose(out=p_tr[:],
                                    in_=t_pall[:, 2 * g:2 * g + 2],
                                    identity=t_id[:])
                nc.vector.tensor_copy(out=t_pT[:, 128 * g:128 * g + 128],
                                      in_=p_tr[:])
            initT_view = t_pT[:].rearrange("p (g q) -> p g q", g=4)[:, :, 0:125]
            nc.sync.dma_start(out=o_init[:], in_=initT_view)

            # ---------- refinement loop ----------
            for step in range(STEPS):
                # ix = clip(((c*2-1+1)*256-1)*0.5, 0, 255), ref op order
                t_u = sb.tile([128, 8], F32, tag="cm1")
                nc.vector.tensor_scalar(out=t_u[:], in0=t_pall[:],
                                        scalar1=2.0, scalar2=1.0,
                                        op0=OP.mult, op1=OP.subtract)
                t_v = sb.tile([128, 8], F32, tag="cm2")
                nc.vector.tensor_scalar(out=t_v[:], in0=t_u[:], scalar1=1.0,
                                        scalar2=None, op0=OP.add)
                t_w = sb.tile([128, 8], F32, tag="cm3")
                nc.vector.tensor_scalar(out=t_w[:], in0=t_v[:],
                                        scalar1=256.0, scalar2=1.0,
                                        op0=OP.mult, op1=OP.subtract)
                t_ix = sb.tile([128, 8], F32, tag="cm4")
                nc.vector.tensor_scalar(out=t_ix[:], in0=t_w[:], scalar1=0.5,
                                        scalar2=None, op0=OP.mult)
                nc.vector.tensor_scalar(out=t_ix[:], in0=t_ix[:],
                                        scalar1=0.0, scalar2=255.0,
                                        op0=OP.max, op1=OP.min)
                t_ri = sb.tile([128, 8], I32, tag="cm5")
                nc.vector.tensor_copy(out=t_ri[:], in_=t_ix[:])
                t_rf = sb.tile([128, 8], F32, tag="cm6")
                nc.vector.tensor_copy(out=t_rf[:], in_=t_ri[:])
                t_gt = sb.tile([128, 8], F32, tag="cm7")
                nc.vector.tensor_tensor(out=t_gt[:], in0=t_rf[:], in1=t_ix[:],
                                        op=OP.is_gt)
                t_fl = sb.tile([128, 8], F32, tag="cm8")
                nc.vector.tensor_tensor(out=t_fl[:], in0=t_rf[:], in1=t_gt[:],
                                        op=OP.subtract)
                t_wf = sb.tile([128, 8], F32, tag="cm9")
                nc.vector.tensor_tensor(out=t_wf[:], in0=t_ix[:], in1=t_fl[:],
                                        op=OP.subtract)
                t_om = sb.tile([128, 8], F32, tag="cm10")
                nc.vector.tensor_scalar(out=t_om[:], in0=t_wf[:],
                                        scalar1=-1.0, scalar2=1.0,
                                        op0=OP.mult, op1=OP.add)
                flv = t_fl[:].rearrange("p (g t) -> p g t", t=2)
                t_idxf = sb.tile([128, 4], F32, tag="cm11")
                nc.vector.tensor_scalar(out=t_idxf[:], in0=flv[:, :, 1],
                                        scalar1=256.0, scalar2=None,
                                        op0=OP.mult)
                nc.vector.tensor_tensor(out=t_idxf[:], in0=t_idxf[:],
                                        in1=flv[:, :, 0], op=OP.add)
                t_idx = sb.tile([128, 4], I32, tag="cm12")
                nc.vector.tensor_copy(out=t_idx[:], in_=t_idxf[:])

                # corner weights cw[p, 4g+c], c in (00,01,10,11):
                # w00=omx*omy, w01=wx*omy, w10=omx*wy, w11=wx*wy
                wfv = t_wf[:].rearrange("p (g t) -> p g t", t=2)
                omv = t_om[:].rearrange("p (g t) -> p g t", t=2)
                t_cw = sb.tile([128, 16], F32, tag="cw")
                cwv = t_cw[:].rearrange("p (g c) -> p c g", c=4)
                for c, (xp, yp) in enumerate(((omv, omv), (wfv, omv),
                                              (omv, wfv), (wfv, wfv))):
                    nc.vector.tensor_tensor(out=cwv[:, c, :],
                                            in0=xp[:, :, 0], in1=yp[:, :, 1],
                                            op=OP.mult)
                inpT_a = sb.tile([128, 512], F32, tag="inpa")
                inpT_b = sb.tile([128, 512], F32, tag="inpb")
                for g in range(NPG):
                    t_g = gpool.tile([128, 1024], F32, tag="gath")
                    nc.gpsimd.indirect_dma_start(
                        out=t_g[:], out_offset=None, in_=d_patch[:],
                        in_offset=bass.IndirectOffsetOnAxis(
                            ap=t_idx[:, g:g + 1], axis=0))
                    # m_c = f_c * w_c ; sampled = (m0+m1) + (m2+m3)
                    t_m = gpool.tile([128, 1024], F32, tag="m")
                    for blk in range(4):
                        nc.vector.tensor_scalar(
                            out=t_m[:, 256 * blk:256 * blk + 256],
                            in0=t_g[:, 256 * blk:256 * blk + 256],
                            scalar1=t_cw[:, 4 * g + blk:4 * g + blk + 1],
                            scalar2=None, op0=OP.mult)
                    t_tb = gpool.tile([128, 512], F32, tag="tb")
                    nc.vector.tensor_tensor(
                        out=t_tb[:].rearrange("p (a b) -> p a b", a=2),
                        in0=t_m[:].rearrange("p (a b) -> p a b", a=2)[:, :, 0:256],
                        in1=t_m[:].rearrange("p (a b) -> p a b", a=2)[:, :, 256:512],
                        op=OP.add)
                    t_samp = gpool.tile([128, 256], F32, tag="samp")
                    nc.vector.tensor_tensor(out=t_samp[:], in0=t_tb[:, 0:256],
                                            in1=t_tb[:, 256:512], op=OP.add)
                    for hc, dest in ((0, inpT_a), (1, inpT_b)):
                        p_tr = ppt.tile([128, 128], F32, space="PSUM",
                                        tag="tr")
                        nc.tensor.transpose(
                            out=p_tr[:],
                            in_=t_samp[:, 128 * hc:128 * hc + 128],
                            identity=t_id[:])
                        nc.vector.tensor_copy(
                            out=dest[:, 128 * g:128 * g + 128], in_=p_tr[:])

                # mm1: r1T (2 M-chunks of [128, 512])
                r1Ts = []
                for mh, (r1tag, rb1) in enumerate((("r1Ta", t_rb1a),
                                                   ("r1Tb", t_rb1b))):
                    p_r1 = pp.tile([128, 512], F32, space="PSUM", tag="mm")
                    nc.tensor.matmul(out=p_r1[:],
                                     lhsT=t_rw1a[:, 128 * mh:128 * mh + 128],
                                     rhs=inpT_a[:], start=True, stop=False)
                    nc.tensor.matmul(out=p_r1[:],
                                     lhsT=t_rw1b[:, 128 * mh:128 * mh + 128],
                                     rhs=inpT_b[:], start=False, stop=False)
                    nc.tensor.matmul(out=p_r1[:],
                                     lhsT=t_rw1c[:, 128 * mh:128 * mh + 128],
                                     rhs=t_pT[:], start=False, stop=True)
                    r1T = sb.tile([128, 512], F32, tag=r1tag)
                    nc.scalar.activation(r1T[:], p_r1[:], AF.Relu,
                                         bias=rb1[:, :1])
                    r1Ts.append(r1T)
                # mm2: r2T [128, 512]
                p_r2 = pp.tile([128, 512], F32, space="PSUM", tag="mm")
                nc.tensor.matmul(out=p_r2[:], lhsT=t_rw2a[:], rhs=r1Ts[0][:],
                                 start=True, stop=False)
                nc.tensor.matmul(out=p_r2[:], lhsT=t_rw2b[:], rhs=r1Ts[1][:],
                                 start=False, stop=True)
                r2T = sb.tile([128, 512], F32, tag="r2T")
                nc.scalar.activation(r2T[:], p_r2[:], AF.Relu,
                                     bias=t_rb2[:, :1])
                # mm3: disp [2, 512]
                p_r3 = pp.tile([2, 512], F32, space="PSUM", tag="mm")
                nc.tensor.matmul(out=p_r3[:], lhsT=t_rw3[:], rhs=r2T[:],
                                 start=True, stop=True)
                t_th = sb.tile([2, 512], F32, tag="th")
                nc.scalar.activation(t_th[:], p_r3[:], AF.Tanh,
                                     bias=t_rb3[:, :1])
                t_disp = sb.tile([2, 512], F32, tag="disp")
                nc.vector.tensor_scalar(out=t_disp[:], in0=t_th[:],
                                        scalar1=SCALE, scalar2=None,
                                        op0=OP.mult)
                # transpose disp to point layout, update p_all on DVE
                # (fast path for next step's coord math)
                t_dp = sb.tile([128, 8], F32, tag="dp")
                for g in range(NPG):
                    p_tr = ppt.tile([128, 2], F32, space="PSUM", tag="tr")
                    nc.tensor.transpose(
                        out=p_tr[:], in_=t_disp[:, 128 * g:128 * g + 128],
                        identity=t_id[0:2, 0:2])
                    nc.scalar.activation(t_dp[:, 2 * g:2 * g + 2], p_tr[:],
                                         AF.Copy)
                nc.vector.tensor_tensor(out=t_pall[:], in0=t_pall[:],
                                        in1=t_dp[:], op=OP.add)
                nc.vector.tensor_scalar(out=t_pall[:], in0=t_pall[:],
                                        scalar1=0.0, scalar2=1.0,
                                        op0=OP.max, op1=OP.min)
                # rebuild pT (exact transposed copy of p_all) off DVE path
                for g in range(NPG):
                    p_tr2 = ppt.tile([2, 128], F32, space="PSUM", tag="tr")
                    nc.tensor.transpose(out=p_tr2[:],
                                        in_=t_pall[:, 2 * g:2 * g + 2],
                                        identity=t_id[:])
                    nc.scalar.activation(t_pT[:, 128 * g:128 * g + 128],
                                         p_tr2[:], AF.Copy)

            # ---------- outputs ----------
            pT_view = t_pT[:].rearrange("p (g q) -> p g q", g=4)[:, :, 0:125]
            nc.sync.dma_start(out=o_poly[:], in_=pT_view)
            nc.sync.dma_start(out=scratch[:], in_=pT_view)
            # validity: polyfT [100, 10] via c-major bounce + transpose
            t_pf = sb.tile([10, 100], F32)
            nc.sync.dma_start(
                out=t_pf[:].rearrange("p (c n) -> p c n", c=2),
                in_=scratch[:].rearrange("c (p n) -> p c n", p=10))
            p_pfT = ppt.tile([100, 10], F32, space="PSUM", tag="tr")
            nc.tensor.transpose(out=p_pfT[:], in_=t_pf[:],
                                identity=t_id[0:10, 0:10])
            t_pfT = sb.tile([100, 10], F32)
            nc.vector.tensor_copy(out=t_pfT[:], in_=p_pfT[:])
            p_v1 = pp.tile([128, 10], F32, space="PSUM", tag="mm")
            nc.tensor.matmul(out=p_v1[:], lhsT=t_vw1[:], rhs=t_pfT[:],
                             start=True, stop=False)
            nc.tensor.matmul(out=p_v1[:], lhsT=t_vb1[:1, :],
                             rhs=t_ones[:1, 0:10], start=False, stop=True)
            t_v1 = sb.tile([128, 10], F32)
            nc.scalar.activation(t_v1[:], p_v1[:], AF.Relu)
            p_v2 = pp.tile([1, 10], F32, space="PSUM", tag="mm")
            nc.tensor.matmul(out=p_v2[:], lhsT=t_vw2[:], rhs=t_v1[:],
                             start=True, stop=False)
            nc.tensor.matmul(out=p_v2[:], lhsT=t_vb2[:1, :],
                             rhs=t_ones[:1, 0:10], start=False, stop=True)
            t_val = sb.tile([1, 10], F32)
            nc.scalar.activation(t_val[:], p_v2[:], AF.Sigmoid)
            nc.sync.dma_start(out=o_val[:], in_=t_val[:])
    nc.compile()
    return nc


# --------------------------------------------------------------------------
# host-side layout helpers (pure data movement, no arithmetic)
# --------------------------------------------------------------------------
def _build_patch_tables(p2):
    """p2 [4, C, H, W] -> per-batch [65536, 1024] f32:
    row (y*W+x) = [f(y,x), f(y,xc), f(yc,x), f(yc,xc)] channels-last."""
    out = []
    for b in range(B):
        hwc = np.ascontiguousarray(p2[b].transpose(1, 2, 0))  # [H, W, C]
        xc = np.concatenate([hwc[:, 1:, :], hwc[:, -1:, :]], axis=1)
        yc = np.concatenate([hwc[1:, :, :], hwc[-1:, :, :]], axis=0)
        ycxc = np.concatenate([xc[1:, :, :], xc[-1:, :, :]], axis=0)
        tab = np.concatenate([hwc, xc, yc, ycxc], axis=2)  # [H, W, 1024]
        out.append(np.ascontiguousarray(tab.reshape(NPIX, 1024)))
    return out


def _chunk_rows(a, p=128):
    """[K, N] -> [p, (K//p)*N], partition-major chunks for matmul operands."""
    K, N = a.shape
    c = K // p
    return np.ascontiguousarray(
        a.reshape(c, p, N).transpose(1, 0, 2).reshape(p, c * N))


def kernel(**inputs):
    f32 = lambda k: np.asarray(inputs[k], np.float32)
    p2, p4 = f32("p2"), f32("p4")
    iw1, ib1, iw2, ib2, iw3, ib3 = (f32(k) for k in
                                    ("iw1", "ib1", "iw2", "ib2", "iw3", "ib3"))
    rw1, rb1, rw2, rb2, rw3, rb3 = (f32(k) for k in
                                    ("rw1", "rb1", "rw2", "rb2", "rw3", "rb3"))
    vw1, vb1, vw2, vb2 = (f32(k) for k in ("vw1", "vb1", "vw2", "vb2"))

    if "l1" not in _cache:
        _cache["l1"] = build_l1()
        _cache["l2"] = build_l2()
        _cache["l3"] = build_l3()
    cores = list(range(NCORES))
    exec_times = []

    # ---- L1: pooling ----
    in1 = [{"p4s": np.ascontiguousarray(
        p4[:, 32 * k:32 * k + 32].reshape(128, 4096))} for k in range(NCORES)]
    r1 = bass_utils.run_bass_kernel_spmd(_cache["l1"], in1, core_ids=cores,
                                         **_trace_kw())
    exec_times.append(r1.exec_time_ns)
    pooled = np.concatenate([r1.results[k]["o_pool"].reshape(4, 32, 64)
                             for k in range(NCORES)], axis=1)  # [4,256,64]
    flatT = np.ascontiguousarray(pooled.reshape(4, 16384).T)   # [16384,4]
    flatT_ch = _chunk_rows(flatT)                              # [128,512]

    # ---- L2: h1 slices ----
    in2 = [{
        "flatT_ch": flatT_ch,
        "iw1s_ch": _chunk_rows(np.ascontiguousarray(
            iw1[:, 64 * k:64 * k + 64])),
        "ib1s": np.ascontiguousarray(ib1[64 * k:64 * k + 64].reshape(64, 1)),
    } for k in range(NCORES)]
    r2 = bass_utils.run_bass_kernel_spmd(_cache["l2"], in2, core_ids=cores,
                                         **_trace_kw())
    exec_times.append(r2.exec_time_ns)
    h1T = np.concatenate([r2.results[k]["o_h1"] for k in range(NCORES)],
                         axis=0)                               # [512, 4]
    h1T_ch = _chunk_rows(h1T)                                  # [128, 16]

    # ---- L3: main ----
    patches = _build_patch_tables(p2)
    vw1p = np.ascontiguousarray(
        vw1.reshape(50, 2, 128).transpose(1, 0, 2).reshape(100, 128))
    common = {
        "h1T_ch": h1T_ch,
        "iw2_ch": _chunk_rows(iw2),
        "ib2": ib2.reshape(1, 1024),
        "rw1a": np.ascontiguousarray(rw1[0:128]),
        "rw1b": np.ascontiguousarray(rw1[128:256]),
        "rw1c": np.ascontiguousarray(rw1[256:258]),
        "rb1a": np.ascontiguousarray(rb1[0:128].reshape(128, 1)),
        "rb1b": np.ascontiguousarray(rb1[128:256].reshape(128, 1)),
        "rw2a": np.ascontiguousarray(rw2[0:128]),
        "rw2b": np.ascontiguousarray(rw2[128:256]),
        "rb2T": rb2.reshape(128, 1),
        "rw3": rw3, "rb3T": rb3.reshape(2, 1),
        "vw1p": vw1p, "vb1": vb1.reshape(1, 128),
        "vw2": vw2, "vb2": vb2.reshape(1, 1),
    }
    in3 = []
    for k in range(NCORES):
        b, par = k // 2, k % 2
        m = dict(common)
        m["iw3s_ch"] = _chunk_rows(np.ascontiguousarray(
            iw3[:, 1000 * par:1000 * par + 1000]))
        m["ib3s"] = np.ascontiguousarray(
            ib3[1000 * par:1000 * par + 1000].reshape(1, 1000))
        m["bsel"] = np.eye(4, dtype=np.float32)[:, b:b + 1]
        m["patch"] = patches[b]
        in3.append(m)
    r3 = bass_utils.run_bass_kernel_spmd(_cache["l3"], in3, core_ids=cores,
                                         **_trace_kw())
    exec_times.append(r3.exec_time_ns)

    polygons = np.zeros((B, MAX_P, MAX_N, 2), np.float32)
    validity = np.zeros((B, MAX_P), np.float32)
    init_p = np.zeros((B, MAX_P, MAX_N, 2), np.float32)
    for k in range(NCORES):
        b, par = k // 2, k % 2
        o = r3.results[k]
        init_p[b, 10 * par:10 * par + 10] = \
            np.ascontiguousarray(o["o_init"].T).reshape(10, 50, 2)
        polygons[b, 10 * par:10 * par + 10] = \
            np.ascontiguousarray(o["o_poly"].T).reshape(10, 50, 2)
        validity[b, 10 * par:10 * par + 10] = o["o_val"][0]

    kernel.last_exec_times = exec_times
    kernel.last_results = (r1, r2, r3)
    return polygons, validity, init_p


kernel.last_exec_times = []
kernel.last_results = None
_TRACE = {"on": False}


def _trace_kw():
    return {"trace": True} if _TRACE["on"] else {}


def enable_trace():
    """Used by test.py; requires the NTFF hook (see hwprof)."""
    _TRACE["on"] = True


# revision 11
# speedup vs baseline: 1.0603x; 1.0294x over previous
"""Trainium2 Bass kernel for nn_DifferentiableVectorization (8 NeuronCores).

Strategy (no collectives -- measured ~41-90us each on this runner):
  3 SPMD launches with host-side LAYOUT-ONLY glue (concat/transpose/reshape).
  L1: pool p4 channel-shard      -> pooled shard  [128(b,c), 64] per core
  L2: h1 column-shard            -> relu(flat @ iw1[:,64cols]+ib1) as [64,4]
  L3: h2, h3-slice, poly-sigmoid init, 3 grid-sample refinement steps with
      indirect-DMA 4KB patch gathers, validity MLP.

Sharding: core k in 0..7 -> batch b=k//2, point-half par=k%2 (125*4 points,
  processed as 4 groups of 125 padded to 128 partitions).
p2 is staged per batch as a patch table P[65536, 1024] f32 where row
  (y*256+x) = [f[y,x,:], f[y,xc,:], f[yc,x,:], f[yc,xc,:]] (channels-last,
  xc=min(x+1,255), yc=min(y+1,255)) -- border clamp baked in, one 4KB
  gather per sampled point. Everything fp32: grid-sample on a randn field
  amplifies coordinate error ~3e4x over 3 steps, so no low-precision
  shortcuts anywhere upstream of coordinates.
"""
import numpy as np

import concourse.bacc as bacc
import concourse.bass as bass
import concourse.mybir as mybir
import concourse.tile as tile
from concourse import bass_utils
from concourse.masks import make_identity

F32 = mybir.dt.float32
I32 = mybir.dt.int32
AF = mybir.ActivationFunctionType
OP = mybir.AluOpType

NCORES = 8
B, C, H, W = 4, 256, 256, 256
MAX_P, MAX_N = 20, 50
SCALE = 0.08
STEPS = 3
NPIX = H * W
NPG = 4          # point groups per core
GP = 125         # points per group (125*4 = 500 = half a batch's points)

# Taylor coefficients of sigmoid(x)-0.5 (odd); |x|<0.5 -> err < 1e-9
SIG_C1 = 0.25
SIG_C3 = -1.0 / 48.0
SIG_C5 = 1.0 / 480.0
SIG_C7 = -17.0 / 80640.0
SIG_C9 = 31.0 / 1451520.0

_cache = {}


# --------------------------------------------------------------------------
# L1: pooling of p4 channel shard.  in: p4s [128, 4096]  out: pool [128, 64]
# --------------------------------------------------------------------------
def build_l1():
    nc = bacc.Bacc("TRN2", target_bir_lowering=False, debug=False,
                   num_devices=NCORES)
    d_p4 = nc.dram_tensor("p4s", [128, 4096], F32, kind="ExternalInput")
    o_pool = nc.dram_tensor("o_pool", [128, 64], F32, kind="ExternalOutput")
    with tile.TileContext(nc) as tc:
        with tc.tile_pool(name="sb", bufs=1) as sb:
            t = sb.tile([128, 4096], F32)
            nc.sync.dma_start(out=t[:], in_=d_p4[:])
            t_pool = sb.tile([128, 64], F32)
            v = t[:].rearrange("p (y0 yi x0 xi) -> p y0 x0 yi xi",
                               y0=8, yi=8, x0=8, xi=8)
            nc.vector.tensor_reduce(
                out=t_pool[:].rearrange("p (a b) -> p a b", a=8, b=8),
                in_=v, op=OP.add, axis=mybir.AxisListType.XY)
            t_poolm = sb.tile([128, 64], F32)
            nc.vector.tensor_scalar_mul(out=t_poolm[:], in0=t_pool[:],
                                        scalar1=1.0 / 64.0)
            nc.sync.dma_start(out=o_pool[:], in_=t_poolm[:])
    nc.compile()
    return nc


# --------------------------------------------------------------------------
# L2: h1 column shard. in: flatT_ch [128, 512] (=[128,(c128,4b)]),
#     iw1s_ch [128, 8192] (=[128,(c128,64m)]), ib1s [64, 1]
# out: h1T slice [64, 4] (post-relu)
# --------------------------------------------------------------------------
def build_l2():
    nc = bacc.Bacc("TRN2", target_bir_lowering=False, debug=False,
                   num_devices=NCORES)
    d_flat = nc.dram_tensor("flatT_ch", [128, 512], F32, kind="ExternalInput")
    d_iw1 = nc.dram_tensor("iw1s_ch", [128, 8192], F32, kind="ExternalInput")
    d_ib1 = nc.dram_tensor("ib1s", [64, 1], F32, kind="ExternalInput")
    o_h1 = nc.dram_tensor("o_h1", [64, 4], F32, kind="ExternalOutput")
    with tile.TileContext(nc) as tc:
        with tc.tile_pool(name="sb", bufs=1) as sb, \
             tc.tile_pool(name="ps", bufs=1, space="PSUM") as pp:
            t_flat = sb.tile([128, 512], F32)
            nc.sync.dma_start(out=t_flat[:], in_=d_flat[:])
            t_w = sb.tile([128, 8192], F32)
            nc.sync.dma_start(out=t_w[:], in_=d_iw1[:])
            t_b = sb.tile([64, 1], F32)
            nc.sync.dma_start(out=t_b[:], in_=d_ib1[:])
            t_id2 = sb.tile([4, 4], F32)
            make_identity(nc, t_id2[:])
            p_h1 = pp.tile([4, 64], F32, space="PSUM")
            for c in range(128):
                nc.tensor.matmul(out=p_h1[:],
                                 lhsT=t_flat[:, 4 * c:4 * c + 4],
                                 rhs=t_w[:, 64 * c:64 * c + 64],
                                 start=(c == 0), stop=(c == 127))
            t_h1p = sb.tile([4, 64], F32)
            nc.vector.tensor_copy(out=t_h1p[:], in_=p_h1[:])
            p_h1T = pp.tile([64, 4], F32, space="PSUM", tag="tr")
            nc.tensor.transpose(out=p_h1T[:], in_=t_h1p[:],
                                identity=t_id2[0:4, 0:4])
            t_h1 = sb.tile([64, 4], F32)
            nc.scalar.activation(t_h1[:], p_h1T[:], AF.Relu, bias=t_b[:, :1])
            nc.sync.dma_start(out=o_h1[:], in_=t_h1[:])
    nc.compile()
    return nc


# --------------------------------------------------------------------------
# L3: the main kernel (per core: batch b = pid//2, half par = pid%2;
#     batch selection via host-supplied one-hot "bsel")
# --------------------------------------------------------------------------
def build_l3():
    nc = bacc.Bacc("TRN2", target_bir_lowering=False, debug=False,
                   num_devices=NCORES)
    d_h1 = nc.dram_tensor("h1T_ch", [128, 16], F32, kind="ExternalInput")
    d_iw2 = nc.dram_tensor("iw2_ch", [128, 4096], F32, kind="ExternalInput")
    d_ib2 = nc.dram_tensor("ib2", [1, 1024], F32, kind="ExternalInput")
    d_iw3 = nc.dram_tensor("iw3s_ch", [128, 8000], F32, kind="ExternalInput")
    d_ib3 = nc.dram_tensor("ib3s", [1, 1000], F32, kind="ExternalInput")
    d_rw1a = nc.dram_tensor("rw1a", [128, 256], F32, kind="ExternalInput")
    d_rw1b = nc.dram_tensor("rw1b", [128, 256], F32, kind="ExternalInput")
    d_rw1c = nc.dram_tensor("rw1c", [2, 256], F32, kind="ExternalInput")
    d_rb1a = nc.dram_tensor("rb1a", [128, 1], F32, kind="ExternalInput")
    d_rb1b = nc.dram_tensor("rb1b", [128, 1], F32, kind="ExternalInput")
    d_rw2a = nc.dram_tensor("rw2a", [128, 128], F32, kind="ExternalInput")
    d_rw2b = nc.dram_tensor("rw2b", [128, 128], F32, kind="ExternalInput")
    d_rb2 = nc.dram_tensor("rb2T", [128, 1], F32, kind="ExternalInput")
    d_rw3 = nc.dram_tensor("rw3", [128, 2], F32, kind="ExternalInput")
    d_rb3 = nc.dram_tensor("rb3T", [2, 1], F32, kind="ExternalInput")
    d_vw1 = nc.dram_tensor("vw1p", [100, 128], F32, kind="ExternalInput")
    d_vb1 = nc.dram_tensor("vb1", [1, 128], F32, kind="ExternalInput")
    d_vw2 = nc.dram_tensor("vw2", [128, 1], F32, kind="ExternalInput")
    d_vb2 = nc.dram_tensor("vb2", [1, 1], F32, kind="ExternalInput")
    d_bsel = nc.dram_tensor("bsel", [4, 1], F32, kind="ExternalInput")
    d_patch = nc.dram_tensor("patch", [NPIX, 1024], F32, kind="ExternalInput")

    o_init = nc.dram_tensor("o_init", [2, 500], F32, kind="ExternalOutput")
    o_poly = nc.dram_tensor("o_poly", [2, 500], F32, kind="ExternalOutput")
    o_val = nc.dram_tensor("o_val", [1, 10], F32, kind="ExternalOutput")

    scratch = nc.dram_tensor("scratch_poly", [2, 500], F32)

    with tile.TileContext(nc) as tc:
        with tc.tile_pool(name="sb", bufs=1) as sb, \
             tc.tile_pool(name="gpool", bufs=4) as gpool, \
             tc.tile_pool(name="ps", bufs=2, space="PSUM") as pp, \
             tc.tile_pool(name="pst", bufs=2, space="PSUM") as ppt:

            # ---------- loads ----------
            def load(name, dram, shape):
                t = sb.tile(shape, F32, tag=name)
                nc.sync.dma_start(out=t[:], in_=dram[:])
                return t

            t_h1 = load("h1", d_h1, [128, 16])
            t_iw2 = load("iw2", d_iw2, [128, 4096])
            t_ib2 = load("ib2", d_ib2, [1, 1024])
            t_iw3 = load("iw3", d_iw3, [128, 8000])
            t_ib3 = load("ib3", d_ib3, [1, 1000])
            t_rw1a = load("rw1a", d_rw1a, [128, 256])
            t_rw1b = load("rw1b", d_rw1b, [128, 256])
            t_rw1c = load("rw1c", d_rw1c, [2, 256])
            t_rb1a = load("rb1a", d_rb1a, [128, 1])
            t_rb1b = load("rb1b", d_rb1b, [128, 1])
            t_rw2a = load("rw2a", d_rw2a, [128, 128])
            t_rw2b = load("rw2b", d_rw2b, [128, 128])
            t_rb2 = load("rb2", d_rb2, [128, 1])
            t_rw3 = load("rw3", d_rw3, [128, 2])
            t_rb3 = load("rb3", d_rb3, [2, 1])
            t_vw1 = load("vw1", d_vw1, [100, 128])
            t_vb1 = load("vb1", d_vb1, [1, 128])
            t_vw2 = load("vw2", d_vw2, [128, 1])
            t_vb2 = load("vb2", d_vb2, [1, 1])
            t_bsel = load("bsel", d_bsel, [4, 1])
            t_ones = sb.tile([1, 1024], F32)
            nc.vector.memset(t_ones[:], 1.0)
            t_id = sb.tile([128, 128], F32)
            make_identity(nc, t_id[:])

            # ---------- h2 = relu(h1 @ iw2 + ib2) ----------
            iw2v = t_iw2[:].rearrange("p (c n) -> p c n", c=4)
            p_h2a = pp.tile([4, 512], F32, space="PSUM", tag="mm")
            p_h2b = pp.tile([4, 512], F32, space="PSUM", tag="mm")
            for nh, p_h2 in ((0, p_h2a), (1, p_h2b)):
                for kc in range(4):
                    nc.tensor.matmul(
                        out=p_h2[:],
                        lhsT=t_h1[:, 4 * kc:4 * kc + 4],
                        rhs=iw2v[:, kc, 512 * nh:512 * nh + 512],
                        start=(kc == 0), stop=False)
                nc.tensor.matmul(out=p_h2[:], lhsT=t_ones[:1, :4],
                                 rhs=t_ib2[:1, 512 * nh:512 * nh + 512],
                                 start=False, stop=True)
            t_h2 = sb.tile([4, 1024], F32)
            nc.scalar.activation(t_h2[:, 0:512], p_h2a[:], AF.Relu)
            nc.scalar.activation(t_h2[:, 512:1024], p_h2b[:], AF.Relu)
            # h2T [128, 32] = 8 transposes of [4,128]
            t_h2T = sb.tile([128, 32], F32)
            for c in range(8):
                p_tr = ppt.tile([128, 4], F32, space="PSUM", tag="tr")
                nc.tensor.transpose(out=p_tr[:],
                                    in_=t_h2[:, 128 * c:128 * c + 128],
                                    identity=t_id[0:4, 0:4])
                nc.vector.tensor_copy(out=t_h2T[:, 4 * c:4 * c + 4],
                                      in_=p_tr[:])

            # ---------- h3 slice + sigmoid (poly) ----------
            iw3v = t_iw3[:].rearrange("p (c n) -> p c n", c=8)
            p_h3a = pp.tile([4, 500], F32, space="PSUM", tag="mm")
            p_h3b = pp.tile([4, 500], F32, space="PSUM", tag="mm")
            for nh, p_h3 in ((0, p_h3a), (1, p_h3b)):
                for kc in range(8):
                    nc.tensor.matmul(
                        out=p_h3[:],
                        lhsT=t_h2T[:, 4 * kc:4 * kc + 4],
                        rhs=iw3v[:, kc, 500 * nh:500 * nh + 500],
                        start=(kc == 0), stop=False)
                nc.tensor.matmul(out=p_h3[:], lhsT=t_ones[:1, :4],
                                 rhs=t_ib3[:1, 500 * nh:500 * nh + 500],
                                 start=False, stop=True)
            # ---------- batch-select h3 preact, transpose to point layout,
            # sigmoid poly on [128, 8] ----------
            t_h3 = sb.tile([4, 1000], F32)
            nc.vector.tensor_copy(out=t_h3[:, 0:500], in_=p_h3a[:])
            nc.vector.tensor_copy(out=t_h3[:, 500:1000], in_=p_h3b[:])
            t_pre = sb.tile([1, 1024], F32)
            nc.vector.memset(t_pre[:], 0.0)
            for nh in range(2):
                p_sel = ppt.tile([1, 500], F32, space="PSUM", tag="tr")
                nc.tensor.matmul(out=p_sel[:], lhsT=t_bsel[:],
                                 rhs=t_h3[:, 500 * nh:500 * nh + 500],
                                 start=True, stop=True)
                nc.vector.tensor_copy(out=t_pre[:, 500 * nh:500 * nh + 500],
                                      in_=p_sel[:])
            prev = t_pre[:].rearrange("o (q t) -> o q t", t=2)
            t_prep = sb.tile([128, 8], F32)
            nc.vector.memset(t_prep[:], 0.0)
            for g in range(NPG):
                for cxy in range(2):
                    p_tr = ppt.tile([128, 1], F32, space="PSUM", tag="tr")
                    nc.tensor.transpose(
                        out=p_tr[0:125, :],
                        in_=prev[:, 125 * g:125 * g + 125, cxy],
                        identity=t_id[0:1, 0:1])
                    nc.vector.tensor_copy(
                        out=t_prep[0:125, 2 * g + cxy:2 * g + cxy + 1],
                        in_=p_tr[0:125, :])
            # sigmoid(x) = 0.5 + x*P(x^2); sigmoid(0) = 0.5 handles pad rows
            t_pall = sb.tile([128, 8], F32)
            t_x2 = sb.tile([128, 8], F32)
            nc.vector.tensor_tensor(out=t_x2[:], in0=t_prep[:], in1=t_prep[:],
                                    op=OP.mult)
            t_p = sb.tile([128, 8], F32)
            nc.vector.tensor_scalar(out=t_p[:], in0=t_x2[:], scalar1=SIG_C9,
                                    scalar2=SIG_C7, op0=OP.mult, op1=OP.add)
            for cc in (SIG_C5, SIG_C3, SIG_C1):
                nc.vector.tensor_tensor(out=t_p[:], in0=t_p[:], in1=t_x2[:],
                                        op=OP.mult)
                nc.vector.tensor_scalar(out=t_p[:], in0=t_p[:], scalar1=cc,
                                        scalar2=None, op0=OP.add)
            nc.vector.tensor_tensor(out=t_p[:], in0=t_p[:], in1=t_prep[:],
                                    op=OP.mult)
            nc.vector.tensor_scalar(out=t_pall[:], in0=t_p[:],
                                    scalar1=0.5, scalar2=None, op0=OP.add)
            # pT [2, 512] from p_all; o_init = pT points (host de-interleaves)
            t_pT = sb.tile([2, 512], F32)
            nc.vector.memset(t_pT[:], 0.5)
            for g in range(NPG):
                p_tr = ppt.tile([2, 128], F32, space="PSUM", tag="tr")
                nc.tensor.transpose(out=p_tr[:],
                                    in_=t_pall[:, 2 * g:2 * g + 2],
                                    identity=t_id[:])
                nc.vector.tensor_copy(out=t_pT[:, 128 * g:128 * g + 128],
                                      in_=p_tr[:])
            initT_view = t_pT[:].rearrange("p (g q) -> p g q", g=4)[:, :, 0:125]
            nc.sync.dma_start(out=o_init[:], in_=initT_view)

            # ---------- refinement loop ----------
            for step in range(STEPS):
                # ix = clip(((c*2-1+1)*256-1)*0.5, 0, 255), ref op order
                t_u = sb.tile([128, 8], F32, tag="cm1")
                nc.vector.tensor_scalar(out=t_u[:], in0=t_pall[:],
                                        scalar1=2.0, scalar2=1.0,
                                        op0=OP.mult, op1=OP.subtract)
                t_v = sb.tile([128, 8], F32, tag="cm2")
                nc.vector.tensor_scalar(out=t_v[:], in0=t_u[:], scalar1=1.0,
                                        scalar2=None, op0=OP.add)
                t_w = sb.tile([128, 8], F32, tag="cm3")
                nc.vector.tensor_scalar(out=t_w[:], in0=t_v[:],
                                        scalar1=256.0, scalar2=1.0,
                                        op0=OP.mult, op1=OP.subtract)
                t_ix = sb.tile([128, 8], F32, tag="cm4")
                nc.vector.tensor_scalar(out=t_ix[:], in0=t_w[:], scalar1=0.5,
                                        scalar2=None, op0=OP.mult)
                nc.vector.tensor_scalar(out=t_ix[:], in0=t_ix[:],
                                        scalar1=0.0, scalar2=255.0,
                                        op0=OP.max, op1=OP.min)
                t_ri = sb.tile([128, 8], I32, tag="cm5")
                nc.vector.tensor_copy(out=t_ri[:], in_=t_ix[:])
                t_rf = sb.tile([128, 8], F32, tag="cm6")
                nc.vector.tensor_copy(out=t_rf[:], in_=t_ri[:])
                t_gt = sb.tile([128, 8], F32, tag="cm7")
                nc.vector.tensor_tensor(out=t_gt[:], in0=t_rf[:], in1=t_ix[:],
                                        op=OP.is_gt)
                t_fl = sb.tile([128, 8], F32, tag="cm8")
                nc.vector.tensor_tensor(out=t_fl[:], in0=t_rf[:], in1=t_gt[:],
                                        op=OP.subtract)
                t_wf = sb.tile([128, 8], F32, tag="cm9")
                nc.vector.tensor_tensor(out=t_wf[:], in0=t_ix[:], in1=t_fl[:],
                                        op=OP.subtract)
                t_om = sb.tile([128, 8], F32, tag="cm10")
                nc.vector.tensor_scalar(out=t_om[:], in0=t_wf[:],
                                        scalar1=-1.0, scalar2=1.0,
                                        op0=OP.mult, op1=OP.add)
                flv = t_fl[:].rearrange("p (g t) -> p g t", t=2)
                t_idxf = sb.tile([128, 4], F32, tag="cm11")
                nc.vector.tensor_scalar(out=t_idxf[:], in0=flv[:, :, 1],
                                        scalar1=256.0, scalar2=None,
                                        op0=OP.mult)
                nc.vector.tensor_tensor(out=t_idxf[:], in0=t_idxf[:],
                                        in1=flv[:, :, 0], op=OP.add)
                t_idx = sb.tile([128, 4], I32, tag="cm12")
                nc.vector.tensor_copy(out=t_idx[:], in_=t_idxf[:])

                # corner weights cw[p, 4g+c], c in (00,01,10,11):
                # w00=omx*omy, w01=wx*omy, w10=omx*wy, w11=wx*wy
                wfv = t_wf[:].rearrange("p (g t) -> p g t", t=2)
                omv = t_om[:].rearrange("p (g t) -> p g t", t=2)
                t_cw = sb.tile([128, 16], F32, tag="cw")
                cwv = t_cw[:].rearrange("p (g c) -> p c g", c=4)
                for c, (xp, yp) in enumerate(((omv, omv), (wfv, omv),
                                              (omv, wfv), (wfv, wfv))):
                    nc.vector.tensor_tensor(out=cwv[:, c, :],
                                            in0=xp[:, :, 0], in1=yp[:, :, 1],
                                            op=OP.mult)
                inpT_a = sb.tile([128, 512], F32, tag="inpa")
                inpT_b = sb.tile([128, 512], F32, tag="inpb")
                for g in range(NPG):
                    t_g = gpool.tile([128, 1024], F32, tag="gath")
                    nc.gpsimd.indirect_dma_start(
                        out=t_g[:], out_offset=None, in_=d_patch[:],
                        in_offset=bass.IndirectOffsetOnAxis(
                            ap=t_idx[:, g:g + 1], axis=0))
                    # m_c = f_c * w_c ; sampled = (m0+m1) + (m2+m3)
                    t_m = gpool.tile([128, 1024], F32, tag="m")
                    for blk in range(4):
                        nc.vector.tensor_scalar(
                            out=t_m[:, 256 * blk:256 * blk + 256],
                            in0=t_g[:, 256 * blk:256 * blk + 256],
                            scalar1=t_cw[:, 4 * g + blk:4 * g + blk + 1],
                            scalar2=None, op0=OP.mult)
                    t_tb = gpool.tile([128, 512], F32, tag="tb")
                    nc.vector.tensor_tensor(
                        out=t_tb[:].rearrange("p (a b) -> p a b", a=2),
                        in0=t_m[:].rearrange("p (a b) -> p a b", a=2)[:, :, 0:256],
                        in1=t_m[:].rearrange("p (a b) -> p a b", a=2)[:, :, 256:512],
                        op=OP.add)
                    t_samp = gpool.tile([128, 256], F32, tag="samp")
                    nc.vector.tensor_tensor(out=t_samp[:], in0=t_tb[:, 0:256],
                                            in1=t_tb[:, 256:512], op=OP.add)
                    for hc, dest in ((0, inpT_a), (1, inpT_b)):
                        p_tr = ppt.tile([128, 128], F32, space="PSUM",
                                        tag="tr")
                        nc.tensor.transpose(
                            out=p_tr[:],
                            in_=t_samp[:, 128 * hc:128 * hc + 128],
                            identity=t_id[:])
                        nc.vector.tensor_copy(
                            out=dest[:, 128 * g:128 * g + 128], in_=p_tr[:])

                # mm1: r1T (2 M-chunks of [128, 512])
                r1Ts = []
                for mh, (r1tag, rb1) in enumerate((("r1Ta", t_rb1a),
                                                   ("r1Tb", t_rb1b))):
                    p_r1 = pp.tile([128, 512], F32, space="PSUM", tag="mm")
                    nc.tensor.matmul(out=p_r1[:],
                                     lhsT=t_rw1a[:, 128 * mh:128 * mh + 128],
                                     rhs=inpT_a[:], start=True, stop=False)
                    nc.tensor.matmul(out=p_r1[:],
                                     lhsT=t_rw1b[:, 128 * mh:128 * mh + 128],
                                     rhs=inpT_b[:], start=False, stop=False)
                    nc.tensor.matmul(out=p_r1[:],
                                     lhsT=t_rw1c[:, 128 * mh:128 * mh + 128],
                                     rhs=t_pT[:], start=False, stop=True)
                    r1T = sb.tile([128, 512], F32, tag=r1tag)
                    nc.scalar.activation(r1T[:], p_r1[:], AF.Relu,
                                         bias=rb1[:, :1])
                    r1Ts.append(r1T)
                # mm2: r2T [128, 512]
                p_r2 = pp.tile([128, 512], F32, space="PSUM", tag="mm")
                nc.tensor.matmul(out=p_r2[:], lhsT=t_rw2a[:], rhs=r1Ts[0][:],
                                 start=True, stop=False)
                nc.tensor.matmul(out=p_r2[:], lhsT=t_rw2b[:], rhs=r1Ts[1][:],
                                 start=False, stop=True)
                r2T = sb.tile([128, 512], F32, tag="r2T")
                nc.scalar.activation(r2T[:], p_r2[:], AF.Relu,
                                     bias=t_rb2[:, :1])
                # mm3: disp [2, 512]
                p_r3 = pp.tile([2, 512], F32, space="PSUM", tag="mm")
                nc.tensor.matmul(out=p_r3[:], lhsT=t_rw3[:], rhs=r2T[:],
                                 start=True, stop=True)
                t_th = sb.tile([2, 512], F32, tag="th")
                nc.scalar.activation(t_th[:], p_r3[:], AF.Tanh,
                                     bias=t_rb3[:, :1])
                t_disp = sb.tile([2, 512], F32, tag="disp")
                nc.vector.tensor_scalar(out=t_disp[:], in0=t_th[:],
                                        scalar1=SCALE, scalar2=None,
                                        op0=OP.mult)
                # transpose disp to point layout, update p_all on DVE
                # (fast path for next step's coord math)
                t_dp = sb.tile([128, 8], F32, tag="dp")
                p_trd = ppt.tile([128, 8], F32, space="PSUM", tag="trd")
                for g in range(NPG):
                    nc.tensor.transpose(
                        out=p_trd[:, 2 * g:2 * g + 2],
                        in_=t_disp[:, 128 * g:128 * g + 128],
                        identity=t_id[0:2, 0:2])
                nc.vector.tensor_copy(out=t_dp[:], in_=p_trd[:])
                nc.vector.tensor_tensor(out=t_pall[:], in0=t_pall[:],
                                        in1=t_dp[:], op=OP.add)
                nc.vector.tensor_scalar(out=t_pall[:], in0=t_pall[:],
                                        scalar1=0.0, scalar2=1.0,
                                        op0=OP.max, op1=OP.min)
                # rebuild pT (exact transposed copy of p_all) off DVE path
                p_trT = ppt.tile([2, 512], F32, space="PSUM", tag="trT")
                for g in range(NPG):
                    nc.tensor.transpose(out=p_trT[:, 128 * g:128 * g + 128],
                                        in_=t_pall[:, 2 * g:2 * g + 2],
                                        identity=t_id[:])
                nc.scalar.activation(t_pT[:], p_trT[:], AF.Copy)

            # ---------- outputs ----------
            pT_view = t_pT[:].rearrange("p (g q) -> p g q", g=4)[:, :, 0:125]
            nc.sync.dma_start(out=o_poly[:], in_=pT_view)
            nc.sync.dma_start(out=scratch[:], in_=pT_view)
            # validity: polyfT [100, 10] via c-major bounce + transpose
            t_pf = sb.tile([10, 100], F32)
            nc.sync.dma_start(
                out=t_pf[:].rearrange("p (c n) -> p c n", c=2),
                in_=scratch[:].rearrange("c (p n) -> p c n", p=10))
            p_pfT = ppt.tile([100, 10], F32, space="PSUM", tag="tr")
            nc.tensor.transpose(out=p_pfT[:], in_=t_pf[:],
                                identity=t_id[0:10, 0:10])
            t_pfT = sb.tile([100, 10], F32)
            nc.vector.tensor_copy(out=t_pfT[:], in_=p_pfT[:])
            p_v1 = pp.tile([128, 10], F32, space="PSUM", tag="mm")
            nc.tensor.matmul(out=p_v1[:], lhsT=t_vw1[:], rhs=t_pfT[:],
                             start=True, stop=False)
            nc.tensor.matmul(out=p_v1[:], lhsT=t_vb1[:1, :],
                             rhs=t_ones[:1, 0:10], start=False, stop=True)
            t_v1 = sb.tile([128, 10], F32)
            nc.scalar.activation(t_v1[:], p_v1[:], AF.Relu)
            p_v2 = pp.tile([1, 10], F32, space="PSUM", tag="mm")
            nc.tensor.matmul(out=p_v2[:], lhsT=t_vw2[:], rhs=t_v1[:],
                             start=True, stop=False)
            nc.tensor.matmul(out=p_v2[:], lhsT=t_vb2[:1, :],
                             rhs=t_ones[:1, 0:10], start=False, stop=True)
            t_val = sb.tile([1, 10], F32)
            nc.scalar.activation(t_val[:], p_v2[:], AF.Sigmoid)
            nc.sync.dma_start(out=o_val[:], in_=t_val[:])
    nc.compile()
    return nc


# --------------------------------------------------------------------------
# host-side layout helpers (pure data movement, no arithmetic)
# --------------------------------------------------------------------------
def _build_patch_tables(p2):
    """p2 [4, C, H, W] -> per-batch [65536, 1024] f32:
    row (y*W+x) = [f(y,x), f(y,xc), f(yc,x), f(yc,xc)] channels-last."""
    out = []
    for b in range(B):
        hwc = np.ascontiguousarray(p2[b].transpose(1, 2, 0))  # [H, W, C]
        xc = np.concatenate([hwc[:, 1:, :], hwc[:, -1:, :]], axis=1)
        yc = np.concatenate([hwc[1:, :, :], hwc[-1:, :, :]], axis=0)
        ycxc = np.concatenate([xc[1:, :, :], xc[-1:, :, :]], axis=0)
        tab = np.concatenate([hwc, xc, yc, ycxc], axis=2)  # [H, W, 1024]
        out.append(np.ascontiguousarray(tab.reshape(NPIX, 1024)))
    return out


def _chunk_rows(a, p=128):
    """[K, N] -> [p, (K//p)*N], partition-major chunks for matmul operands."""
    K, N = a.shape
    c = K // p
    return np.ascontiguousarray(
        a.reshape(c, p, N).transpose(1, 0, 2).reshape(p, c * N))


def kernel(**inputs):
    f32 = lambda k: np.asarray(inputs[k], np.float32)
    p2, p4 = f32("p2"), f32("p4")
    iw1, ib1, iw2, ib2, iw3, ib3 = (f32(k) for k in
                                    ("iw1", "ib1", "iw2", "ib2", "iw3", "ib3"))
    rw1, rb1, rw2, rb2, rw3, rb3 = (f32(k) for k in
                                    ("rw1", "rb1", "rw2", "rb2", "rw3", "rb3"))
    vw1, vb1, vw2, vb2 = (f32(k) for k in ("vw1", "vb1", "vw2", "vb2"))

    if "l1" not in _cache:
        _cache["l1"] = build_l1()
        _cache["l2"] = build_l2()
        _cache["l3"] = build_l3()
    cores = list(range(NCORES))
    exec_times = []

    # ---- L1: pooling ----
    in1 = [{"p4s": np.ascontiguousarray(
        p4[:, 32 * k:32 * k + 32].reshape(128, 4096))} for k in range(NCORES)]
    r1 = bass_utils.run_bass_kernel_spmd(_cache["l1"], in1, core_ids=cores,
                                         **_trace_kw())
    exec_times.append(r1.exec_time_ns)
    pooled = np.concatenate([r1.results[k]["o_pool"].reshape(4, 32, 64)
                             for k in range(NCORES)], axis=1)  # [4,256,64]
    flatT = np.ascontiguousarray(pooled.reshape(4, 16384).T)   # [16384,4]
    flatT_ch = _chunk_rows(flatT)                              # [128,512]

    # ---- L2: h1 slices ----
    in2 = [{
        "flatT_ch": flatT_ch,
        "iw1s_ch": _chunk_rows(np.ascontiguousarray(
            iw1[:, 64 * k:64 * k + 64])),
        "ib1s": np.ascontiguousarray(ib1[64 * k:64 * k + 64].reshape(64, 1)),
    } for k in range(NCORES)]
    r2 = bass_utils.run_bass_kernel_spmd(_cache["l2"], in2, core_ids=cores,
                                         **_trace_kw())
    exec_times.append(r2.exec_time_ns)
    h1T = np.concatenate([r2.results[k]["o_h1"] for k in range(NCORES)],
                         axis=0)                               # [512, 4]
    h1T_ch = _chunk_rows(h1T)                                  # [128, 16]

    # ---- L3: main ----
    patches = _build_patch_tables(p2)
    vw1p = np.ascontiguousarray(
        vw1.reshape(50, 2, 128).transpose(1, 0, 2).reshape(100, 128))
    common = {
        "h1T_ch": h1T_ch,
        "iw2_ch": _chunk_rows(iw2),
        "ib2": ib2.reshape(1, 1024),
        "rw1a": np.ascontiguousarray(rw1[0:128]),
        "rw1b": np.ascontiguousarray(rw1[128:256]),
        "rw1c": np.ascontiguousarray(rw1[256:258]),
        "rb1a": np.ascontiguousarray(rb1[0:128].reshape(128, 1)),
        "rb1b": np.ascontiguousarray(rb1[128:256].reshape(128, 1)),
        "rw2a": np.ascontiguousarray(rw2[0:128]),
        "rw2b": np.ascontiguousarray(rw2[128:256]),
        "rb2T": rb2.reshape(128, 1),
        "rw3": rw3, "rb3T": rb3.reshape(2, 1),
        "vw1p": vw1p, "vb1": vb1.reshape(1, 128),
        "vw2": vw2, "vb2": vb2.reshape(1, 1),
    }
    in3 = []
    for k in range(NCORES):
        b, par = k // 2, k % 2
        m = dict(common)
        m["iw3s_ch"] = _chunk_rows(np.ascontiguousarray(
            iw3[:, 1000 * par:1000 * par + 1000]))
        m["ib3s"] = np.ascontiguousarray(
            ib3[1000 * par:1000 * par + 1000].reshape(1, 1000))
        m["bsel"] = np.eye(4, dtype=np.float32)[:, b:b + 1]
        m["patch"] = patches[b]
        in3.append(m)
    r3 = bass_utils.run_bass_kernel_spmd(_cache["l3"], in3, core_ids=cores,
                                         **_trace_kw())
    exec_times.append(r3.exec_time_ns)

    polygons = np.zeros((B, MAX_P, MAX_N, 2), np.float32)
    validity = np.zeros((B, MAX_P), np.float32)
    init_p = np.zeros((B, MAX_P, MAX_N, 2), np.float32)
    for k in range(NCORES):
        b, par = k // 2, k % 2
        o = r3.results[k]
        init_p[b, 10 * par:10 * par + 10] = \
            np.ascontiguousarray(o["o_init"].T).reshape(10, 50, 2)
        polygons[b, 10 * par:10 * par + 10] = \
            np.ascontiguousarray(o["o_poly"].T).reshape(10, 50, 2)
        validity[b, 10 * par:10 * par + 10] = o["o_val"][0]

    kernel.last_exec_times = exec_times
    kernel.last_results = (r1, r2, r3)
    return polygons, validity, init_p


kernel.last_exec_times = []
kernel.last_results = None
_TRACE = {"on": False}


def _trace_kw():
    return {"trace": True} if _TRACE["on"] else {}


def enable_trace():
    """Used by test.py; requires the NTFF hook (see hwprof)."""
    _TRACE["on"] = True


# revision 12
# speedup vs baseline: 1.1140x; 1.0506x over previous
"""Trainium2 Bass kernel for nn_DifferentiableVectorization (8 NeuronCores).

Strategy (no collectives -- measured ~41-90us each on this runner):
  3 SPMD launches with host-side LAYOUT-ONLY glue (concat/transpose/reshape).
  L1: pool p4 channel-shard      -> pooled shard  [128(b,c), 64] per core
  L2: h1 column-shard            -> relu(flat @ iw1[:,64cols]+ib1) as [64,4]
  L3: h2, h3-slice, poly-sigmoid init, 3 grid-sample refinement steps with
      indirect-DMA 4KB patch gathers, validity MLP.

Sharding: core k in 0..7 -> batch b=k//2, point-half par=k%2 (125*4 points,
  processed as 4 groups of 125 padded to 128 partitions).
p2 is staged per batch as a patch table P[65536, 1024] f32 where row
  (y*256+x) = [f[y,x,:], f[y,xc,:], f[yc,x,:], f[yc,xc,:]] (channels-last,
  xc=min(x+1,255), yc=min(y+1,255)) -- border clamp baked in, one 4KB
  gather per sampled point. Everything fp32: grid-sample on a randn field
  amplifies coordinate error ~3e4x over 3 steps, so no low-precision
  shortcuts anywhere upstream of coordinates.
"""
import numpy as np

import concourse.bacc as bacc
import concourse.bass as bass
import concourse.mybir as mybir
import concourse.tile as tile
from concourse import bass_utils
from concourse.masks import make_identity

F32 = mybir.dt.float32
I32 = mybir.dt.int32
AF = mybir.ActivationFunctionType
OP = mybir.AluOpType

NCORES = 8
B, C, H, W = 4, 256, 256, 256
MAX_P, MAX_N = 20, 50
SCALE = 0.08
STEPS = 3
NPIX = H * W
NPG = 4          # point groups per core
GP = 125         # points per group (125*4 = 500 = half a batch's points)

# Taylor coefficients of sigmoid(x)-0.5 (odd); |x|<0.5 -> err < 1e-9
SIG_C1 = 0.25
SIG_C3 = -1.0 / 48.0
SIG_C5 = 1.0 / 480.0
SIG_C7 = -17.0 / 80640.0
SIG_C9 = 31.0 / 1451520.0

_cache = {}


# --------------------------------------------------------------------------
# L1: pooling of p4 channel shard.  in: p4s [128, 4096]  out: pool [128, 64]
# --------------------------------------------------------------------------
def build_l1():
    nc = bacc.Bacc("TRN2", target_bir_lowering=False, debug=False,
                   num_devices=NCORES)
    d_p4 = nc.dram_tensor("p4s", [128, 4096], F32, kind="ExternalInput")
    o_pool = nc.dram_tensor("o_pool", [128, 64], F32, kind="ExternalOutput")
    with tile.TileContext(nc) as tc:
        with tc.tile_pool(name="sb", bufs=1) as sb:
            t = sb.tile([128, 4096], F32)
            nc.sync.dma_start(out=t[:], in_=d_p4[:])
            t_pool = sb.tile([128, 64], F32)
            v = t[:].rearrange("p (y0 yi x0 xi) -> p y0 x0 yi xi",
                               y0=8, yi=8, x0=8, xi=8)
            nc.vector.tensor_reduce(
                out=t_pool[:].rearrange("p (a b) -> p a b", a=8, b=8),
                in_=v, op=OP.add, axis=mybir.AxisListType.XY)
            t_poolm = sb.tile([128, 64], F32)
            nc.vector.tensor_scalar_mul(out=t_poolm[:], in0=t_pool[:],
                                        scalar1=1.0 / 64.0)
            nc.sync.dma_start(out=o_pool[:], in_=t_poolm[:])
    nc.compile()
    return nc


# --------------------------------------------------------------------------
# L2: h1 column shard. in: flatT_ch [128, 512] (=[128,(c128,4b)]),
#     iw1s_ch [128, 8192] (=[128,(c128,64m)]), ib1s [64, 1]
# out: h1T slice [64, 4] (post-relu)
# --------------------------------------------------------------------------
def build_l2():
    nc = bacc.Bacc("TRN2", target_bir_lowering=False, debug=False,
                   num_devices=NCORES)
    d_flat = nc.dram_tensor("flatT_ch", [128, 512], F32, kind="ExternalInput")
    d_iw1 = nc.dram_tensor("iw1s_ch", [128, 8192], F32, kind="ExternalInput")
    d_ib1 = nc.dram_tensor("ib1s", [64, 1], F32, kind="ExternalInput")
    o_h1 = nc.dram_tensor("o_h1", [64, 4], F32, kind="ExternalOutput")
    with tile.TileContext(nc) as tc:
        with tc.tile_pool(name="sb", bufs=1) as sb, \
             tc.tile_pool(name="ps", bufs=1, space="PSUM") as pp:
            t_flat = sb.tile([128, 512], F32)
            nc.sync.dma_start(out=t_flat[:], in_=d_flat[:])
            t_w = sb.tile([128, 8192], F32)
            nc.sync.dma_start(out=t_w[:], in_=d_iw1[:])
            t_b = sb.tile([64, 1], F32)
            nc.sync.dma_start(out=t_b[:], in_=d_ib1[:])
            t_id2 = sb.tile([4, 4], F32)
            make_identity(nc, t_id2[:])
            p_h1 = pp.tile([4, 64], F32, space="PSUM")
            for c in range(128):
                nc.tensor.matmul(out=p_h1[:],
                                 lhsT=t_flat[:, 4 * c:4 * c + 4],
                                 rhs=t_w[:, 64 * c:64 * c + 64],
                                 start=(c == 0), stop=(c == 127))
            t_h1p = sb.tile([4, 64], F32)
            nc.vector.tensor_copy(out=t_h1p[:], in_=p_h1[:])
            p_h1T = pp.tile([64, 4], F32, space="PSUM", tag="tr")
            nc.tensor.transpose(out=p_h1T[:], in_=t_h1p[:],
                                identity=t_id2[0:4, 0:4])
            t_h1 = sb.tile([64, 4], F32)
            nc.scalar.activation(t_h1[:], p_h1T[:], AF.Relu, bias=t_b[:, :1])
            nc.sync.dma_start(out=o_h1[:], in_=t_h1[:])
    nc.compile()
    return nc


# --------------------------------------------------------------------------
# L3: the main kernel (per core: batch b = pid//2, half par = pid%2;
#     batch selection via host-supplied one-hot "bsel")
# --------------------------------------------------------------------------
def build_l3():
    nc = bacc.Bacc("TRN2", target_bir_lowering=False, debug=False,
                   num_devices=NCORES)
    d_h1 = nc.dram_tensor("h1T_ch", [128, 16], F32, kind="ExternalInput")
    d_iw2 = nc.dram_tensor("iw2_ch", [128, 4096], F32, kind="ExternalInput")
    d_ib2 = nc.dram_tensor("ib2", [1, 1024], F32, kind="ExternalInput")
    d_iw3 = nc.dram_tensor("iw3s_ch", [128, 8000], F32, kind="ExternalInput")
    d_ib3 = nc.dram_tensor("ib3s", [1, 1000], F32, kind="ExternalInput")
    d_rw1a = nc.dram_tensor("rw1a", [128, 256], F32, kind="ExternalInput")
    d_rw1b = nc.dram_tensor("rw1b", [128, 256], F32, kind="ExternalInput")
    d_rw1c = nc.dram_tensor("rw1c", [2, 256], F32, kind="ExternalInput")
    d_rb1a = nc.dram_tensor("rb1a", [128, 1], F32, kind="ExternalInput")
    d_rb1b = nc.dram_tensor("rb1b", [128, 1], F32, kind="ExternalInput")
    d_rw2a = nc.dram_tensor("rw2a", [128, 128], F32, kind="ExternalInput")
    d_rw2b = nc.dram_tensor("rw2b", [128, 128], F32, kind="ExternalInput")
    d_rb2 = nc.dram_tensor("rb2T", [128, 1], F32, kind="ExternalInput")
    d_rw3 = nc.dram_tensor("rw3", [128, 2], F32, kind="ExternalInput")
    d_rb3 = nc.dram_tensor("rb3T", [2, 1], F32, kind="ExternalInput")
    d_vw1 = nc.dram_tensor("vw1p", [100, 128], F32, kind="ExternalInput")
    d_vb1 = nc.dram_tensor("vb1", [1, 128], F32, kind="ExternalInput")
    d_vw2 = nc.dram_tensor("vw2", [128, 1], F32, kind="ExternalInput")
    d_vb2 = nc.dram_tensor("vb2", [1, 1], F32, kind="ExternalInput")
    d_bsel = nc.dram_tensor("bsel", [4, 1], F32, kind="ExternalInput")
    d_patch = nc.dram_tensor("patch", [NPIX, 1024], F32, kind="ExternalInput")

    o_init = nc.dram_tensor("o_init", [2, 500], F32, kind="ExternalOutput")
    o_poly = nc.dram_tensor("o_poly", [2, 500], F32, kind="ExternalOutput")
    o_val = nc.dram_tensor("o_val", [1, 10], F32, kind="ExternalOutput")

    scratch = nc.dram_tensor("scratch_poly", [2, 500], F32)

    with tile.TileContext(nc) as tc:
        with tc.tile_pool(name="sb", bufs=1) as sb, \
             tc.tile_pool(name="gpool", bufs=4) as gpool, \
             tc.tile_pool(name="ps", bufs=2, space="PSUM") as pp, \
             tc.tile_pool(name="pst", bufs=2, space="PSUM") as ppt:

            # ---------- loads ----------
            def load(name, dram, shape):
                t = sb.tile(shape, F32, tag=name)
                nc.sync.dma_start(out=t[:], in_=dram[:])
                return t

            t_h1 = load("h1", d_h1, [128, 16])
            t_iw2 = load("iw2", d_iw2, [128, 4096])
            t_ib2 = load("ib2", d_ib2, [1, 1024])
            t_iw3 = load("iw3", d_iw3, [128, 8000])
            t_ib3 = load("ib3", d_ib3, [1, 1000])
            t_rw1a = load("rw1a", d_rw1a, [128, 256])
            t_rw1b = load("rw1b", d_rw1b, [128, 256])
            t_rw1c = load("rw1c", d_rw1c, [2, 256])
            t_rb1a = load("rb1a", d_rb1a, [128, 1])
            t_rb1b = load("rb1b", d_rb1b, [128, 1])
            t_rw2a = load("rw2a", d_rw2a, [128, 128])
            t_rw2b = load("rw2b", d_rw2b, [128, 128])
            t_rb2 = load("rb2", d_rb2, [128, 1])
            t_rw3 = load("rw3", d_rw3, [128, 2])
            t_rb3 = load("rb3", d_rb3, [2, 1])
            t_vw1 = load("vw1", d_vw1, [100, 128])
            t_vb1 = load("vb1", d_vb1, [1, 128])
            t_vw2 = load("vw2", d_vw2, [128, 1])
            t_vb2 = load("vb2", d_vb2, [1, 1])
            t_bsel = load("bsel", d_bsel, [4, 1])
            t_ones = sb.tile([1, 1024], F32)
            nc.vector.memset(t_ones[:], 1.0)
            t_id = sb.tile([128, 128], F32)
            make_identity(nc, t_id[:])

            # ---------- h2 = relu(h1 @ iw2 + ib2) ----------
            iw2v = t_iw2[:].rearrange("p (c n) -> p c n", c=4)
            p_h2a = pp.tile([4, 512], F32, space="PSUM", tag="mm")
            p_h2b = pp.tile([4, 512], F32, space="PSUM", tag="mm")
            for nh, p_h2 in ((0, p_h2a), (1, p_h2b)):
                for kc in range(4):
                    nc.tensor.matmul(
                        out=p_h2[:],
                        lhsT=t_h1[:, 4 * kc:4 * kc + 4],
                        rhs=iw2v[:, kc, 512 * nh:512 * nh + 512],
                        start=(kc == 0), stop=False)
                nc.tensor.matmul(out=p_h2[:], lhsT=t_ones[:1, :4],
                                 rhs=t_ib2[:1, 512 * nh:512 * nh + 512],
                                 start=False, stop=True)
            t_h2 = sb.tile([4, 1024], F32)
            nc.scalar.activation(t_h2[:, 0:512], p_h2a[:], AF.Relu)
            nc.scalar.activation(t_h2[:, 512:1024], p_h2b[:], AF.Relu)
            # h2T [128, 32] = 8 transposes of [4,128]
            t_h2T = sb.tile([128, 32], F32)
            for c in range(8):
                p_tr = ppt.tile([128, 4], F32, space="PSUM", tag="tr")
                nc.tensor.transpose(out=p_tr[:],
                                    in_=t_h2[:, 128 * c:128 * c + 128],
                                    identity=t_id[0:4, 0:4])
                nc.vector.tensor_copy(out=t_h2T[:, 4 * c:4 * c + 4],
                                      in_=p_tr[:])

            # ---------- h3 slice + sigmoid (poly) ----------
            iw3v = t_iw3[:].rearrange("p (c n) -> p c n", c=8)
            p_h3a = pp.tile([4, 500], F32, space="PSUM", tag="mm")
            p_h3b = pp.tile([4, 500], F32, space="PSUM", tag="mm")
            for nh, p_h3 in ((0, p_h3a), (1, p_h3b)):
                for kc in range(8):
                    nc.tensor.matmul(
                        out=p_h3[:],
                        lhsT=t_h2T[:, 4 * kc:4 * kc + 4],
                        rhs=iw3v[:, kc, 500 * nh:500 * nh + 500],
                        start=(kc == 0), stop=False)
                nc.tensor.matmul(out=p_h3[:], lhsT=t_ones[:1, :4],
                                 rhs=t_ib3[:1, 500 * nh:500 * nh + 500],
                                 start=False, stop=True)
            # ---------- batch-select h3 preact, transpose to point layout,
            # sigmoid poly on [128, 8] ----------
            t_h3 = sb.tile([4, 1000], F32)
            nc.vector.tensor_copy(out=t_h3[:, 0:500], in_=p_h3a[:])
            nc.vector.tensor_copy(out=t_h3[:, 500:1000], in_=p_h3b[:])
            t_pre = sb.tile([1, 1024], F32)
            nc.vector.memset(t_pre[:], 0.0)
            for nh in range(2):
                p_sel = ppt.tile([1, 500], F32, space="PSUM", tag="tr")
                nc.tensor.matmul(out=p_sel[:], lhsT=t_bsel[:],
                                 rhs=t_h3[:, 500 * nh:500 * nh + 500],
                                 start=True, stop=True)
                nc.vector.tensor_copy(out=t_pre[:, 500 * nh:500 * nh + 500],
                                      in_=p_sel[:])
            prev = t_pre[:].rearrange("o (q t) -> o q t", t=2)
            t_prep = sb.tile([128, 8], F32)
            nc.vector.memset(t_prep[:], 0.0)
            for g in range(NPG):
                for cxy in range(2):
                    p_tr = ppt.tile([128, 1], F32, space="PSUM", tag="tr")
                    nc.tensor.transpose(
                        out=p_tr[0:125, :],
                        in_=prev[:, 125 * g:125 * g + 125, cxy],
                        identity=t_id[0:1, 0:1])
                    nc.vector.tensor_copy(
                        out=t_prep[0:125, 2 * g + cxy:2 * g + cxy + 1],
                        in_=p_tr[0:125, :])
            # sigmoid(x) = 0.5 + x*P(x^2); sigmoid(0) = 0.5 handles pad rows
            t_pall = sb.tile([128, 8], F32)
            t_x2 = sb.tile([128, 8], F32)
            nc.vector.tensor_tensor(out=t_x2[:], in0=t_prep[:], in1=t_prep[:],
                                    op=OP.mult)
            t_p = sb.tile([128, 8], F32)
            nc.vector.tensor_scalar(out=t_p[:], in0=t_x2[:], scalar1=SIG_C9,
                                    scalar2=SIG_C7, op0=OP.mult, op1=OP.add)
            for cc in (SIG_C5, SIG_C3, SIG_C1):
                nc.vector.tensor_tensor(out=t_p[:], in0=t_p[:], in1=t_x2[:],
                                        op=OP.mult)
                nc.vector.tensor_scalar(out=t_p[:], in0=t_p[:], scalar1=cc,
                                        scalar2=None, op0=OP.add)
            nc.vector.tensor_tensor(out=t_p[:], in0=t_p[:], in1=t_prep[:],
                                    op=OP.mult)
            nc.vector.tensor_scalar(out=t_pall[:], in0=t_p[:],
                                    scalar1=0.5, scalar2=None, op0=OP.add)
            # pT [2, 512] from p_all; o_init = pT points (host de-interleaves)
            t_pT = sb.tile([2, 512], F32)
            nc.vector.memset(t_pT[:], 0.5)
            for g in range(NPG):
                p_tr = ppt.tile([2, 128], F32, space="PSUM", tag="tr")
                nc.tensor.transpose(out=p_tr[:],
                                    in_=t_pall[:, 2 * g:2 * g + 2],
                                    identity=t_id[:])
                nc.vector.tensor_copy(out=t_pT[:, 128 * g:128 * g + 128],
                                      in_=p_tr[:])
            initT_view = t_pT[:].rearrange("p (g q) -> p g q", g=4)[:, :, 0:125]
            nc.sync.dma_start(out=o_init[:], in_=initT_view)

            # ---------- refinement loop ----------
            for step in range(STEPS):
                # ix = clip(((c*2-1+1)*256-1)*0.5, 0, 255), ref op order
                t_u = sb.tile([128, 8], F32, tag="cm1")
                nc.vector.tensor_scalar(out=t_u[:], in0=t_pall[:],
                                        scalar1=2.0, scalar2=1.0,
                                        op0=OP.mult, op1=OP.subtract)
                t_v = sb.tile([128, 8], F32, tag="cm2")
                nc.vector.tensor_scalar(out=t_v[:], in0=t_u[:], scalar1=1.0,
                                        scalar2=None, op0=OP.add)
                t_w = sb.tile([128, 8], F32, tag="cm3")
                nc.vector.tensor_scalar(out=t_w[:], in0=t_v[:],
                                        scalar1=256.0, scalar2=1.0,
                                        op0=OP.mult, op1=OP.subtract)
                t_ix = sb.tile([128, 8], F32, tag="cm4")
                nc.vector.tensor_scalar(out=t_ix[:], in0=t_w[:], scalar1=0.5,
                                        scalar2=None, op0=OP.mult)
                nc.vector.tensor_scalar(out=t_ix[:], in0=t_ix[:],
                                        scalar1=0.0, scalar2=255.0,
                                        op0=OP.max, op1=OP.min)
                t_ri = sb.tile([128, 8], I32, tag="cm5")
                nc.vector.tensor_copy(out=t_ri[:], in_=t_ix[:])
                t_rf = sb.tile([128, 8], F32, tag="cm6")
                nc.vector.tensor_copy(out=t_rf[:], in_=t_ri[:])
                t_gt = sb.tile([128, 8], F32, tag="cm7")
                nc.vector.tensor_tensor(out=t_gt[:], in0=t_rf[:], in1=t_ix[:],
                                        op=OP.is_gt)
                t_fl = sb.tile([128, 8], F32, tag="cm8")
                nc.vector.tensor_tensor(out=t_fl[:], in0=t_rf[:], in1=t_gt[:],
                                        op=OP.subtract)
                t_wf = sb.tile([128, 8], F32, tag="cm9")
                nc.vector.tensor_tensor(out=t_wf[:], in0=t_ix[:], in1=t_fl[:],
                                        op=OP.subtract)
                t_om = sb.tile([128, 8], F32, tag="cm10")
                nc.vector.tensor_scalar(out=t_om[:], in0=t_wf[:],
                                        scalar1=-1.0, scalar2=1.0,
                                        op0=OP.mult, op1=OP.add)
                flv = t_fl[:].rearrange("p (g t) -> p g t", t=2)
                t_idxf = sb.tile([128, 4], F32, tag="cm11")
                nc.vector.tensor_scalar(out=t_idxf[:], in0=flv[:, :, 1],
                                        scalar1=256.0, scalar2=None,
                                        op0=OP.mult)
                nc.vector.tensor_tensor(out=t_idxf[:], in0=t_idxf[:],
                                        in1=flv[:, :, 0], op=OP.add)
                t_idx = sb.tile([128, 4], I32, tag="cm12")
                nc.vector.tensor_copy(out=t_idx[:], in_=t_idxf[:])

                # corner weights cw[p, 4g+c], c in (00,01,10,11):
                # w00=omx*omy, w01=wx*omy, w10=omx*wy, w11=wx*wy
                wfv = t_wf[:].rearrange("p (g t) -> p g t", t=2)
                omv = t_om[:].rearrange("p (g t) -> p g t", t=2)
                t_cw = sb.tile([128, 16], F32, tag="cw")
                cwv = t_cw[:].rearrange("p (g c) -> p c g", c=4)
                for c, (xp, yp) in enumerate(((omv, omv), (wfv, omv),
                                              (omv, wfv), (wfv, wfv))):
                    nc.vector.tensor_tensor(out=cwv[:, c, :],
                                            in0=xp[:, :, 0], in1=yp[:, :, 1],
                                            op=OP.mult)
                # issue all 4 gathers first (gpsimd queue stays unblocked)
                t_gs = []
                for g in range(NPG):
                    t_g = gpool.tile([128, 1024], F32, tag="gath")
                    nc.gpsimd.indirect_dma_start(
                        out=t_g[:], out_offset=None, in_=d_patch[:],
                        in_offset=bass.IndirectOffsetOnAxis(
                            ap=t_idx[:, g:g + 1], axis=0))
                    t_gs.append(t_g)

                inpT_a = sb.tile([128, 512], F32, tag="inpa")
                inpT_b = sb.tile([128, 512], F32, tag="inpb")
                t_th = sb.tile([2, 512], F32, tag="th")
                # pipelined half-steps: groups {2h, 2h+1}, mm on N=256 slice
                for half in range(2):
                    for g in (2 * half, 2 * half + 1):
                        t_g = t_gs[g]
                        # m_c = f_c * w_c ; sampled = (m0+m1) + (m2+m3)
                        t_m = gpool.tile([128, 1024], F32, tag="m")
                        for blk in range(4):
                            nc.vector.tensor_scalar(
                                out=t_m[:, 256 * blk:256 * blk + 256],
                                in0=t_g[:, 256 * blk:256 * blk + 256],
                                scalar1=t_cw[:, 4 * g + blk:4 * g + blk + 1],
                                scalar2=None, op0=OP.mult)
                        t_tb = gpool.tile([128, 512], F32, tag="tb")
                        nc.vector.tensor_tensor(
                            out=t_tb[:].rearrange("p (a b) -> p a b", a=2),
                            in0=t_m[:].rearrange("p (a b) -> p a b", a=2)[:, :, 0:256],
                            in1=t_m[:].rearrange("p (a b) -> p a b", a=2)[:, :, 256:512],
                            op=OP.add)
                        t_samp = gpool.tile([128, 256], F32, tag="samp")
                        nc.gpsimd.tensor_tensor(out=t_samp[:],
                                                in0=t_tb[:, 0:256],
                                                in1=t_tb[:, 256:512],
                                                op=OP.add)
                        for hc, dest in ((0, inpT_a), (1, inpT_b)):
                            p_tr = ppt.tile([128, 128], F32, space="PSUM",
                                            tag="tr")
                            nc.tensor.transpose(
                                out=p_tr[:],
                                in_=t_samp[:, 128 * hc:128 * hc + 128],
                                identity=t_id[:])
                            nc.vector.tensor_copy(
                                out=dest[:, 128 * g:128 * g + 128],
                                in_=p_tr[:])
                    ns = slice(256 * half, 256 * half + 256)
                    # mm1 on this half's columns
                    r1Ts = []
                    for mh, (r1tag, rb1) in enumerate((("r1Ta", t_rb1a),
                                                       ("r1Tb", t_rb1b))):
                        p_r1 = pp.tile([128, 256], F32, space="PSUM",
                                       tag="mm")
                        nc.tensor.matmul(
                            out=p_r1[:],
                            lhsT=t_rw1a[:, 128 * mh:128 * mh + 128],
                            rhs=inpT_a[:, ns], start=True, stop=False)
                        nc.tensor.matmul(
                            out=p_r1[:],
                            lhsT=t_rw1b[:, 128 * mh:128 * mh + 128],
                            rhs=inpT_b[:, ns], start=False, stop=False)
                        nc.tensor.matmul(
                            out=p_r1[:],
                            lhsT=t_rw1c[:, 128 * mh:128 * mh + 128],
                            rhs=t_pT[:, ns], start=False, stop=True)
                        r1T = sb.tile([128, 512], F32, tag=r1tag)
                        nc.scalar.activation(r1T[:, ns], p_r1[:], AF.Relu,
                                             bias=rb1[:, :1])
                        r1Ts.append(r1T)
                    p_r2 = pp.tile([128, 256], F32, space="PSUM", tag="mm")
                    nc.tensor.matmul(out=p_r2[:], lhsT=t_rw2a[:],
                                     rhs=r1Ts[0][:, ns], start=True,
                                     stop=False)
                    nc.tensor.matmul(out=p_r2[:], lhsT=t_rw2b[:],
                                     rhs=r1Ts[1][:, ns], start=False,
                                     stop=True)
                    r2T = sb.tile([128, 512], F32, tag="r2T")
                    nc.scalar.activation(r2T[:, ns], p_r2[:], AF.Relu,
                                         bias=t_rb2[:, :1])
                    p_r3 = pp.tile([2, 256], F32, space="PSUM", tag="mm")
                    nc.tensor.matmul(out=p_r3[:], lhsT=t_rw3[:],
                                     rhs=r2T[:, ns], start=True, stop=True)
                    nc.scalar.activation(t_th[:, ns], p_r3[:], AF.Tanh,
                                         bias=t_rb3[:, :1])
                t_disp = sb.tile([2, 512], F32, tag="disp")
                nc.vector.tensor_scalar(out=t_disp[:], in0=t_th[:],
                                        scalar1=SCALE, scalar2=None,
                                        op0=OP.mult)
                # transpose disp to point layout, update p_all on DVE
                # (fast path for next step's coord math)
                t_dp = sb.tile([128, 8], F32, tag="dp")
                p_trd = ppt.tile([128, 8], F32, space="PSUM", tag="trd")
                for g in range(NPG):
                    nc.tensor.transpose(
                        out=p_trd[:, 2 * g:2 * g + 2],
                        in_=t_disp[:, 128 * g:128 * g + 128],
                        identity=t_id[0:2, 0:2])
                nc.vector.tensor_copy(out=t_dp[:], in_=p_trd[:])
                nc.vector.tensor_tensor(out=t_pall[:], in0=t_pall[:],
                                        in1=t_dp[:], op=OP.add)
                nc.vector.tensor_scalar(out=t_pall[:], in0=t_pall[:],
                                        scalar1=0.0, scalar2=1.0,
                                        op0=OP.max, op1=OP.min)
                # rebuild pT (exact transposed copy of p_all) off DVE path
                p_trT = ppt.tile([2, 512], F32, space="PSUM", tag="trT")
                for g in range(NPG):
                    nc.tensor.transpose(out=p_trT[:, 128 * g:128 * g + 128],
                                        in_=t_pall[:, 2 * g:2 * g + 2],
                                        identity=t_id[:])
                nc.scalar.activation(t_pT[:], p_trT[:], AF.Copy)

            # ---------- outputs ----------
            pT_view = t_pT[:].rearrange("p (g q) -> p g q", g=4)[:, :, 0:125]
            nc.sync.dma_start(out=o_poly[:], in_=pT_view)
            nc.sync.dma_start(out=scratch[:], in_=pT_view)
            # validity: polyfT [100, 10] via c-major bounce + transpose
            t_pf = sb.tile([10, 100], F32)
            nc.sync.dma_start(
                out=t_pf[:].rearrange("p (c n) -> p c n", c=2),
                in_=scratch[:].rearrange("c (p n) -> p c n", p=10))
            p_pfT = ppt.tile([100, 10], F32, space="PSUM", tag="tr")
            nc.tensor.transpose(out=p_pfT[:], in_=t_pf[:],
                                identity=t_id[0:10, 0:10])
            t_pfT = sb.tile([100, 10], F32)
            nc.vector.tensor_copy(out=t_pfT[:], in_=p_pfT[:])
            p_v1 = pp.tile([128, 10], F32, space="PSUM", tag="mm")
            nc.tensor.matmul(out=p_v1[:], lhsT=t_vw1[:], rhs=t_pfT[:],
                             start=True, stop=False)
            nc.tensor.matmul(out=p_v1[:], lhsT=t_vb1[:1, :],
                             rhs=t_ones[:1, 0:10], start=False, stop=True)
            t_v1 = sb.tile([128, 10], F32)
            nc.scalar.activation(t_v1[:], p_v1[:], AF.Relu)
            p_v2 = pp.tile([1, 10], F32, space="PSUM", tag="mm")
            nc.tensor.matmul(out=p_v2[:], lhsT=t_vw2[:], rhs=t_v1[:],
                             start=True, stop=False)
            nc.tensor.matmul(out=p_v2[:], lhsT=t_vb2[:1, :],
                             rhs=t_ones[:1, 0:10], start=False, stop=True)
            t_val = sb.tile([1, 10], F32)
            nc.scalar.activation(t_val[:], p_v2[:], AF.Sigmoid)
            nc.sync.dma_start(out=o_val[:], in_=t_val[:])
    nc.compile()
    return nc


# --------------------------------------------------------------------------
# host-side layout helpers (pure data movement, no arithmetic)
# --------------------------------------------------------------------------
def _build_patch_tables(p2):
    """p2 [4, C, H, W] -> per-batch [65536, 1024] f32:
    row (y*W+x) = [f(y,x), f(y,xc), f(yc,x), f(yc,xc)] channels-last."""
    out = []
    for b in range(B):
        hwc = np.ascontiguousarray(p2[b].transpose(1, 2, 0))  # [H, W, C]
        xc = np.concatenate([hwc[:, 1:, :], hwc[:, -1:, :]], axis=1)
        yc = np.concatenate([hwc[1:, :, :], hwc[-1:, :, :]], axis=0)
        ycxc = np.concatenate([xc[1:, :, :], xc[-1:, :, :]], axis=0)
        tab = np.concatenate([hwc, xc, yc, ycxc], axis=2)  # [H, W, 1024]
        out.append(np.ascontiguousarray(tab.reshape(NPIX, 1024)))
    return out


def _chunk_rows(a, p=128):
    """[K, N] -> [p, (K//p)*N], partition-major chunks for matmul operands."""
    K, N = a.shape
    c = K // p
    return np.ascontiguousarray(
        a.reshape(c, p, N).transpose(1, 0, 2).reshape(p, c * N))


def kernel(**inputs):
    f32 = lambda k: np.asarray(inputs[k], np.float32)
    p2, p4 = f32("p2"), f32("p4")
    iw1, ib1, iw2, ib2, iw3, ib3 = (f32(k) for k in
                                    ("iw1", "ib1", "iw2", "ib2", "iw3", "ib3"))
    rw1, rb1, rw2, rb2, rw3, rb3 = (f32(k) for k in
                                    ("rw1", "rb1", "rw2", "rb2", "rw3", "rb3"))
    vw1, vb1, vw2, vb2 = (f32(k) for k in ("vw1", "vb1", "vw2", "vb2"))

    if "l1" not in _cache:
        _cache["l1"] = build_l1()
        _cache["l2"] = build_l2()
        _cache["l3"] = build_l3()
    cores = list(range(NCORES))
    exec_times = []

    # ---- L1: pooling ----
    in1 = [{"p4s": np.ascontiguousarray(
        p4[:, 32 * k:32 * k + 32].reshape(128, 4096))} for k in range(NCORES)]
    r1 = bass_utils.run_bass_kernel_spmd(_cache["l1"], in1, core_ids=cores,
                                         **_trace_kw())
    exec_times.append(r1.exec_time_ns)
    pooled = np.concatenate([r1.results[k]["o_pool"].reshape(4, 32, 64)
                             for k in range(NCORES)], axis=1)  # [4,256,64]
    flatT = np.ascontiguousarray(pooled.reshape(4, 16384).T)   # [16384,4]
    flatT_ch = _chunk_rows(flatT)                              # [128,512]

    # ---- L2: h1 slices ----
    in2 = [{
        "flatT_ch": flatT_ch,
        "iw1s_ch": _chunk_rows(np.ascontiguousarray(
            iw1[:, 64 * k:64 * k + 64])),
        "ib1s": np.ascontiguousarray(ib1[64 * k:64 * k + 64].reshape(64, 1)),
    } for k in range(NCORES)]
    r2 = bass_utils.run_bass_kernel_spmd(_cache["l2"], in2, core_ids=cores,
                                         **_trace_kw())
    exec_times.append(r2.exec_time_ns)
    h1T = np.concatenate([r2.results[k]["o_h1"] for k in range(NCORES)],
                         axis=0)                               # [512, 4]
    h1T_ch = _chunk_rows(h1T)                                  # [128, 16]

    # ---- L3: main ----
    patches = _build_patch_tables(p2)
    vw1p = np.ascontiguousarray(
        vw1.reshape(50, 2, 128).transpose(1, 0, 2).reshape(100, 128))
    common = {
        "h1T_ch": h1T_ch,
        "iw2_ch": _chunk_rows(iw2),
        "ib2": ib2.reshape(1, 1024),
        "rw1a": np.ascontiguousarray(rw1[0:128]),
        "rw1b": np.ascontiguousarray(rw1[128:256]),
        "rw1c": np.ascontiguousarray(rw1[256:258]),
        "rb1a": np.ascontiguousarray(rb1[0:128].reshape(128, 1)),
        "rb1b": np.ascontiguousarray(rb1[128:256].reshape(128, 1)),
        "rw2a": np.ascontiguousarray(rw2[0:128]),
        "rw2b": np.ascontiguousarray(rw2[128:256]),
        "rb2T": rb2.reshape(128, 1),
        "rw3": rw3, "rb3T": rb3.reshape(2, 1),
        "vw1p": vw1p, "vb1": vb1.reshape(1, 128),
        "vw2": vw2, "vb2": vb2.reshape(1, 1),
    }
    in3 = []
    for k in range(NCORES):
        b, par = k // 2, k % 2
        m = dict(common)
        m["iw3s_ch"] = _chunk_rows(np.ascontiguousarray(
            iw3[:, 1000 * par:1000 * par + 1000]))
        m["ib3s"] = np.ascontiguousarray(
            ib3[1000 * par:1000 * par + 1000].reshape(1, 1000))
        m["bsel"] = np.eye(4, dtype=np.float32)[:, b:b + 1]
        m["patch"] = patches[b]
        in3.append(m)
    r3 = bass_utils.run_bass_kernel_spmd(_cache["l3"], in3, core_ids=cores,
                                         **_trace_kw())
    exec_times.append(r3.exec_time_ns)

    polygons = np.zeros((B, MAX_P, MAX_N, 2), np.float32)
    validity = np.zeros((B, MAX_P), np.float32)
    init_p = np.zeros((B, MAX_P, MAX_N, 2), np.float32)
    for k in range(NCORES):
        b, par = k // 2, k % 2
        o = r3.results[k]
        init_p[b, 10 * par:10 * par + 10] = \
            np.ascontiguousarray(o["o_init"].T).reshape(10, 50, 2)
        polygons[b, 10 * par:10 * par + 10] = \
            np.ascontiguousarray(o["o_poly"].T).reshape(10, 50, 2)
        validity[b, 10 * par:10 * par + 10] = o["o_val"][0]

    kernel.last_exec_times = exec_times
    kernel.last_results = (r1, r2, r3)
    return polygons, validity, init_p


kernel.last_exec_times = []
kernel.last_results = None
_TRACE = {"on": False}


def _trace_kw():
    return {"trace": True} if _TRACE["on"] else {}


def enable_trace():
    """Used by test.py; requires the NTFF hook (see hwprof)."""
    _TRACE["on"] = True
